# revision 47
# baseline (speedup 1.0000x reference)
"""Trainium2 Bass kernel for nn_MaxMarginLoss (segment_reduce).

Data-parallel over the batch: 32 samples -> 8 NeuronCores x 4 samples.

Per core, for each sample b:
  - segment sums over T=2048 timesteps into S=32 step buckets are computed
    on TensorE as mask[128t,32s].T @ |x|[128t,1024d], accumulated over 16
    K-chunks into PSUM (this is the memory-bound part: 32 MiB of `inputs`
    per core, streamed as 2 MiB contiguous DMAs).
  - the appearance-order logic avoids any sort: first-appearance positions
    come from a masked min-reduce; each step's rank is the count of
    strictly-smaller packed keys (pos*33 + id); the ordered-adjacency
    matrix A[i,j] = (rank_j == rank_i + 1 and j present) turns "gather by
    argsort and diff neighbours" into a tiny 32x32 matmul H_next = A @ H.
  - pair energies E_i = mean_d relu(H_i - H_next_i)^2 via Relu + Square
    with fused free-dim accumulation.
Each core returns [4,5] per-sample sums (npairs, n, ninv, sum E*valid,
sum relu(1-E)*inv); the host applies the binary labels and the final
scalar division (a few hundred flops).
"""

import numpy as np

import concourse.bass as bass
from concourse import mybir
from concourse.bass_utils import run_bass_kernel_spmd
from concourse.tile import TileContext
from concourse.vector_clock import ScopedClock

F32 = mybir.dt.float32
BF16 = mybir.dt.bfloat16
U32 = mybir.dt.uint32
U16 = mybir.dt.uint16
I8 = mybir.dt.int8
I16 = mybir.dt.int16
OP = mybir.AluOpType
AF = mybir.ActivationFunctionType

B, T, D = 32, 2048, 1024
S = 32          # step ids 1..32; id 0 is padding
ALPHA = 1.0
N_CORES = 8
BL = B // N_CORES           # samples per core
K = 128                     # matmul contraction tile (partitions)
NCHUNK = T // K             # 16 K-chunks per sample
XT = 2                      # K-chunks per x DMA ([128, XT, 1024] = 1 MiB)

# The public neuronxcc walrus (setupSyncWait in CoreV2/V3GenImpl) only
# supports a small number of embedded semaphore waits per instruction,
# while Tile's scheduler attaches one wait per required logical proc.
# After scheduling, hoist overflow waits onto same-engine no-ops placed
# immediately before the owning instruction: engine program order makes
# that semantically identical.
_MAX_WAITS_DEFAULT = 1
_MAX_WAITS_BY_OPCODE = {}


class _LeanTailTileContext(TileContext):
    """Tile's default kernel tail is drain -> barrier -> sem-clear ->
    barrier.  After the first all-engine barrier no engine can still be
    waiting on a kernel semaphore, so the clears need no cross-engine
    ordering and the second (~3-4 us) barrier can be dropped; each
    engine's stream still ends after its own clears, so re-execution
    sees zeroed semaphores."""

    def _drain_and_barrier(self, tick_clock, wait_clock):
        drain_inst = self.nc.sync.drain()
        wait_clock.add_sem_waits(
            drain_inst.ins, ScopedClock({None: tick_clock.global_clock})
        )
        self.nc.all_engine_barrier()
        assert self.sems is not None
        popped = self.nc._tile_sem_poison_stack.pop()
        assert popped is self._sem_poison
        self.nc.clear_and_free_semaphores(list(self.sems.allocated().values()))


def _split_sync_waits(nc: bass.Bass):
    for f in nc.m.functions:
        for bb in f.blocks:
            insts = list(bb.instructions)
            need = []  # (ins, overflow_waits)
            for ins in insts:
                si = getattr(ins, "sync_info", None)
                if si is None or not si.on_wait:
                    continue
                cap = _MAX_WAITS_BY_OPCODE.get(ins.opcode, _MAX_WAITS_DEFAULT)
                waits = list(si.on_wait)
                if len(waits) <= cap:
                    continue
                ins.sync_info = mybir.SyncInfo(
                    on_wait=waits[:cap], on_update=list(si.on_update)
                )
                need.append((ins, waits[cap:], cap))
            if not need:
                continue
            nop_for: dict[str, list] = {}
            for ins, overflow, cap in need:
                eng = nc.engines[ins.engine]
                nops = []
                for i in range(0, len(overflow), cap):
                    nop = eng.nop(hint="waitsplit", nofuse=True)
                    nop.ins.sync_info = mybir.SyncInfo(
                        on_wait=overflow[i:i + cap], on_update=[]
                    )
                    nops.append(nop.ins)
                nop_for[ins.name] = nops
            created = {n.name for nops in nop_for.values() for n in nops}
            # nop() appended the new instructions to the current bb; pull
            # them out of every block and splice before their owners.
            for bb2 in f.blocks:
                cur = [i for i in bb2.instructions if i.name not in created]
                out = []
                for ins in cur:
                    out.extend(nop_for.get(ins.name, ()))
                    out.append(ins)
                bb2.instructions = out


# column offsets inside the packed int8 index buffer "cst8"
C8_IDSBC = 0                  # [K, T]   step ids, row 32b+s = ids of sample b
C8_IDSREP = C8_IDSBC + T      # [K, BL*NCHUNK*S] mask-layout ids, 32x repeated
C8_IOTAT = C8_IDSREP + BL * NCHUNK * S   # [K, NCHUNK*S] tiled 1..32
CW8 = C8_IOTAT + NCHUNK * S
# column offsets inside the packed float32 constant buffer "cst32"
C_STEPS = 0                   # [K, 1]
C_LOWER = C_STEPS + 1         # [K, S] block [i > j]
C_ONES = C_LOWER + S          # [K, S] ones
C_BONES = C_ONES + S          # [K, BL] block-diagonal ones
CW32 = C_BONES + BL


def build_program() -> bass.Bass:
    nc = bass.Bass()

    x = nc.declare_dram_parameter("x", [BL, T, D], F32, isOutput=False)
    cst8 = nc.declare_dram_parameter("cst8", [K, CW8], I8, isOutput=False)
    tmt16 = nc.declare_dram_parameter("tmt16", [K, T], I16, isOutput=False)
    cst32 = nc.declare_dram_parameter("cst32", [K, CW32], F32, isOutput=False)
    out5 = nc.declare_dram_parameter("out5", [BL, 5], F32, isOutput=True)

    with _LeanTailTileContext(nc) as tc:
        with (
            tc.tile_pool(name="const", bufs=1) as cpool,
            tc.tile_pool(name="persist", bufs=1) as pp,
            tc.tile_pool(name="xin", bufs=6) as xin,
            tc.tile_pool(name="xabs", bufs=6) as xabs,
            tc.tile_pool(name="mk", bufs=2) as mkp,
            tc.tile_pool(name="ps_sums", bufs=2, space="PSUM") as ps_sums,
            tc.tile_pool(name="ps_misc", bufs=1, space="PSUM") as ps_misc,
        ):
            # ---- constants / index data, narrow dtypes, DMA'd via the
            #      (otherwise idle) SWDGE queue so the HWDGE rings start
            #      on x immediately --------------------------------------
            sb_cst8 = cpool.tile([K, CW8], I8)
            nc.gpsimd.dma_start(out=sb_cst8[:], in_=cst8[:])
            sb_tmt = cpool.tile([K, T], I16)
            nc.gpsimd.dma_start(out=sb_tmt[:], in_=tmt16[:])
            sb_cst32 = cpool.tile([K, CW32], F32)
            nc.gpsimd.dma_start(out=sb_cst32[:], in_=cst32[:])
            sb_idsbc = sb_cst8[:, C8_IDSBC:C8_IDSBC + T]
            sb_steps = sb_cst32[:, C_STEPS:C_STEPS + 1]
            sb_lower = sb_cst32[:, C_LOWER:C_LOWER + S]
            sb_ones = sb_cst32[:, C_ONES:C_ONES + S]
            sb_bones = sb_cst32[:, C_BONES:C_BONES + BL]

            # ---- phase A: masks / positions / ranks (all 4 samples
            #      stacked on partitions: row 32*b + s) ------------------
            maskf = pp.tile([K, T], F32)        # [s-stacked, t] 0/1 mask
            counts = pp.tile([K, 1], F32)
            nc.vector.tensor_scalar(
                maskf[:], sb_idsbc[:], sb_steps[:], None, OP.is_equal, OP.add,
                accum_out=counts[:],
            )
            tm = pp.tile([K, T], F32)           # mask * (t - T)
            nc.vector.tensor_tensor(tm[:], maskf[:], sb_tmt[:], OP.mult)
            posm = pp.tile([K, 1], F32)         # pos - T (present) else 0
            nc.vector.tensor_reduce(posm[:], tm[:], mybir.AxisListType.X, OP.min)

            cnt1 = pp.tile([K, 1], F32)
            nc.vector.tensor_scalar(cnt1[:], counts[:], 1.0, None, OP.max)
            recip = pp.tile([K, 1], F32)        # 1 / max(counts, 1)
            nc.vector.reciprocal(recip[:], cnt1[:])

            # distinct sort keys: (pos-T)*33 + (s+1); order == stable
            # argsort of pos with id tiebreak (present strictly first)
            key = pp.tile([K, 1], F32)
            nc.vector.tensor_scalar(
                key[:], posm[:], 33.0, sb_steps[:], OP.mult, OP.add
            )
            key_sq = pp.tile([K, S], F32)
            nc.vector.tensor_scalar(key_sq[:], sb_ones[:], key[:], None, OP.mult)
            key_t = pp.tile([K, S], F32)        # row i holds key_l along l
            nc.vector.transpose(key_t[:], key_sq[:])
            cmp = pp.tile([K, S], F32)
            rank = pp.tile([K, 1], F32)
            nc.vector.tensor_scalar(
                cmp[:], key_t[:], key[:], None, OP.is_lt, OP.add,
                accum_out=rank[:],
            )
            rankp1 = pp.tile([K, 1], F32)
            nc.vector.tensor_scalar(rankp1[:], rank[:], 1.0, None, OP.add)
            t999 = pp.tile([K, 1], F32)         # 999 for absent steps
            nc.vector.tensor_scalar(
                t999[:], posm[:], 0.0, 999.0, OP.is_ge, OP.mult
            )
            rankp = pp.tile([K, 1], F32)        # rank, pushed out if absent
            nc.vector.tensor_tensor(rankp[:], rank[:], t999[:], OP.add)

            v_t = pp.tile([K, 8], F32)          # per-step stats columns
            nc.vector.tensor_scalar(v_t[:, 1:2], posm[:], 0.0, None, OP.is_lt)

            rankp_sq = pp.tile([K, S], F32)
            nc.vector.tensor_scalar(rankp_sq[:], sb_ones[:], rankp[:], None, OP.mult)
            rankp_t = pp.tile([K, S], F32)
            nc.vector.transpose(rankp_t[:], rankp_sq[:])
            rankp1_sq = pp.tile([K, S], F32)
            nc.vector.tensor_scalar(rankp1_sq[:], sb_ones[:], rankp1[:], None, OP.mult)
            rankp1_t = pp.tile([K, S], F32)
            nc.vector.transpose(rankp1_t[:], rankp1_sq[:])

            # A[i,j] = (rankp_j == rank_i + 1); succ_i = sum_j A[i,j]
            a_m = pp.tile([K, S], F32)
            nc.vector.tensor_scalar(
                a_m[:], rankp_t[:], rankp1[:], None, OP.is_equal, OP.add,
                accum_out=v_t[:, 0:1],
            )
            # A^T (lhsT for the H_next matmul; 0/1 so bf16 is exact)
            a_t = pp.tile([K, S], BF16)
            nc.vector.tensor_scalar(
                a_t[:], rankp1_t[:], rankp[:], None, OP.is_equal
            )
            # inv_i = sum_j A[i,j] * [i > j]
            a_inv = pp.tile([K, S], F32)
            nc.vector.scalar_tensor_tensor(
                a_inv[:], rankp_t[:], rankp1[:], sb_lower[:],
                op0=OP.is_equal, op1=OP.mult, accum_out=v_t[:, 2:3],
            )

            # ---- phase B: segment sums via TensorE --------------------
            h_all = pp.tile([K, D], BF16)
            hn = ps_misc.tile([K, D], F32)      # 2 PSUM banks
            diff = pp.tile([K, D], F32)
            sq = pp.tile([K, D], F32)
            e_raw = pp.tile([K, 1], F32)
            ps_of = {}

            def sample_tail(b):
                ps_all = ps_of[b]
                # H, H_next and pair energies for sample b; emitted one
                # sample late (mid-stream of sample b+1) so these ops land
                # in each engine's in-order queue at a point where their
                # dependencies are already met — emitted eagerly they
                # head-of-line-block the abs stream and stall the DMAs
                bs = slice(b * S, (b + 1) * S)
                for h in range(2):
                    nc.vector.tensor_scalar(
                        h_all[bs, h * 512:(h + 1) * 512],
                        ps_all[bs, h * 512:(h + 1) * 512],
                        recip[bs], None, OP.mult,
                    )
                for h in range(2):
                    nc.tensor.matmul(
                        hn[bs, h * 512:(h + 1) * 512],
                        lhsT=a_t[bs, :],
                        rhs=h_all[bs, h * 512:(h + 1) * 512],
                        start=True, stop=True,
                        tile_position=(b * S, b * S),
                    )
                nc.vector.tensor_tensor(
                    diff[bs, :], h_all[bs, :], hn[bs, :], OP.subtract
                )
                # relu(d)^2 == max(d,0)*d, with the free-dim sum fused in
                nc.vector.scalar_tensor_tensor(
                    sq[bs, :], diff[bs, :], 0.0, diff[bs, :],
                    op0=OP.max, op1=OP.mult, accum_out=e_raw[bs, :],
                )

            for b in range(BL):
                # all 16 mask chunks of the sample in one compare against
                # host-replicated ids (layout matches the x DMA below)
                mk_all = mkp.tile([K, NCHUNK * S], BF16)
                nc.vector.tensor_tensor(
                    mk_all[:],
                    sb_cst8[:, C8_IDSREP + b * NCHUNK * S:
                            C8_IDSREP + (b + 1) * NCHUNK * S],
                    sb_cst8[:, C8_IOTAT:C8_IOTAT + NCHUNK * S],
                    OP.is_equal,
                )
                # fresh PSUM banks per sample: sample b+1 accumulates while
                # sample b's H-scale still reads its own banks (no WAR)
                ps_all = ps_sums.tile([K, D], F32, tag="ps")
                ps_of[b] = ps_all
                for tq in range(NCHUNK // XT):
                    if tq == (NCHUNK // XT) // 2 and b > 0:
                        sample_tail(b - 1)
                    ti = b * (NCHUNK // XT) + tq
                    xt = xin.tile([K, XT, D], F32)
                    # alternate the two HWDGE rings (sync / scalar) so x
                    # streaming is not serialized on a single ring.
                    # Partition p holds XT consecutive T-rows (16 KiB of
                    # contiguous DRAM per partition); any (partition, sub)
                    # <-> t bijection works for the contraction as long as
                    # the host-side ids layout matches.
                    dma_eng = nc.sync if ti % 2 == 0 else nc.scalar
                    dma_eng.dma_start(
                        out=xt[:],
                        in_=x[b, tq * XT * K:(tq + 1) * XT * K, :].rearrange(
                            "(p s) d -> p s d", p=K
                        ),
                    )
                    # |x| rounded to bf16: the PE runs bf16 at 1 cycle/row
                    # vs fp32's 4; the 2^-9 relative rounding on |x| washes
                    # out to ~1e-4 in the final loss (mask stays exact 0/1).
                    # Alternate engines: ACT computes Abs->bf16 directly; DVE
                    # rounds to bf16 (RNE, so |bf16(x)| == bf16(|x|)) then
                    # clears the sign bit in place in the 16-bit 4x mode.
                    xa = xabs.tile([K, XT, D], BF16)
                    if ti % 2 == 0:
                        nc.scalar.activation(xa[:], xt[:], AF.Abs)
                    else:
                        nc.vector.tensor_copy(xa[:], xt[:])
                        nc.vector.tensor_scalar(
                            xa[:].bitcast(U16), xa[:].bitcast(U16),
                            0x7FFF, None, OP.bitwise_and,
                        )
                    for sub in range(XT):
                        c = tq * XT + sub
                        for h in range(2):
                            nc.tensor.matmul(
                                ps_all[b * S:(b + 1) * S, h * 512:(h + 1) * 512],
                                lhsT=mk_all[:, c * S:(c + 1) * S],
                                rhs=xa[:, sub, h * 512:(h + 1) * 512],
                                start=(c == 0), stop=(c == NCHUNK - 1),
                                tile_position=(0, b * S),
                            )
            sample_tail(BL - 1)

            # ---- phase C: combine per-step stats ----------------------
            e_col = pp.tile([K, 1], F32)
            nc.vector.tensor_scalar(e_col[:], e_raw[:], 1.0 / D, None, OP.mult)
            nc.vector.tensor_tensor(v_t[:, 3:4], e_col[:], v_t[:, 0:1], OP.mult)
            ae1 = pp.tile([K, 1], F32)          # relu(ALPHA - E)
            nc.vector.tensor_scalar(
                ae1[:], e_col[:], -1.0, ALPHA, OP.mult, OP.add
            )
            ae = pp.tile([K, 1], F32)
            nc.vector.tensor_scalar(ae[:], ae1[:], 0.0, None, OP.max)
            nc.vector.tensor_tensor(v_t[:, 4:5], ae[:], v_t[:, 2:3], OP.mult)

            # per-sample column sums: blockones[128,4].T @ V[128,5] -> [4,5]
            vp = ps_misc.tile([BL, 8], F32)
            nc.tensor.matmul(
                vp[:, 0:5], lhsT=sb_bones[:], rhs=v_t[:, 0:5],
                start=True, stop=True,
            )
            out_sb = pp.tile([BL, 5], F32)
            nc.vector.tensor_copy(out_sb[:], vp[:, 0:5])
            nc.sync.dma_start(out=out5[:], in_=out_sb[:])

    _split_sync_waits(nc)
    return nc


_PROGRAM: bass.Bass | None = None


def get_program() -> bass.Bass:
    global _PROGRAM
    if _PROGRAM is None:
        _PROGRAM = build_program()
    return _PROGRAM


def make_in_maps(inputs: np.ndarray, step_ids: np.ndarray) -> list[dict]:
    """Shard + pre-layout the (tiny) index tensors per core."""
    inputs = np.ascontiguousarray(np.asarray(inputs, dtype=np.float32))
    step_ids = np.asarray(step_ids)

    tmt16 = np.tile(
        (np.arange(T) - T).astype(np.int16)[None, :], (K, 1)
    )
    iota_t = np.tile(
        np.tile(np.arange(1, S + 1, dtype=np.int8), NCHUNK)[None, :], (K, 1)
    )
    cst32 = np.empty((K, CW32), dtype=np.float32)
    cst32[:, C_STEPS:C_STEPS + 1] = np.tile(
        np.arange(1, S + 1, dtype=np.float32), BL
    )[:, None]
    cst32[:, C_LOWER:C_LOWER + S] = np.tile(
        (np.arange(S)[:, None] > np.arange(S)[None, :]).astype(np.float32),
        (BL, 1),
    )
    cst32[:, C_ONES:C_ONES + S] = 1.0
    cst32[:, C_BONES:C_BONES + BL] = (
        (np.arange(K)[:, None] // S) == np.arange(BL)[None, :]
    ).astype(np.float32)

    in_maps = []
    for core in range(N_CORES):
        b0 = core * BL
        ids = step_ids[b0:b0 + BL].astype(np.int8)              # [4, 2048]
        # matmul chunk (b, tq, sub) contracts t = tq*XT*K + p*XT + sub on
        # partition p; idsrep repeats each id S times along the free dim so
        # one is_equal against iota_t yields all NCHUNK mask chunks
        idsrep = np.repeat(
            ids.reshape(BL, NCHUNK // XT, K, XT).transpose(2, 0, 1, 3)
            .reshape(K, BL, NCHUNK),
            S, axis=2,
        ).reshape(K, BL * NCHUNK * S)
        cst8 = np.empty((K, CW8), dtype=np.int8)
        cst8[:, C8_IDSBC:C8_IDSBC + T] = np.repeat(ids, S, axis=0)
        cst8[:, C8_IDSREP:C8_IDSREP + BL * NCHUNK * S] = idsrep
        cst8[:, C8_IOTAT:C8_IOTAT + NCHUNK * S] = iota_t
        in_maps.append({
            "x": inputs[b0:b0 + BL],
            "cst8": cst8,
            "tmt16": tmt16,
            "cst32": cst32,
        })
    return in_maps


def finish_host(out5_per_core: list[np.ndarray], binary_labels: np.ndarray):
    """Combine per-sample (npairs, n, ninv, S1, S2) with labels."""
    v = np.concatenate([np.asarray(o, np.float64) for o in out5_per_core], axis=0)
    npairs, n, ninv, s1, s2 = v[:, 0], v[:, 1], v[:, 2], v[:, 3], v[:, 4]
    labels = np.asarray(binary_labels)
    loss_pos = s1 / np.maximum(npairs, 1.0)
    loss_neg = s2 / np.maximum(ninv, 1.0)
    pos_count = (labels == 1) & (n >= 2)
    neg_count = (labels == 0) & (ninv > 0)
    total = (loss_pos * pos_count).sum() + (loss_neg * neg_count).sum()
    num = pos_count.sum() + neg_count.sum()
    return np.float32(total / (num + 1e-9))


def kernel(inputs, step_ids, binary_labels, _trace=False):
    nc = get_program()
    in_maps = make_in_maps(inputs, step_ids)
    res = run_bass_kernel_spmd(
        nc, in_maps, core_ids=list(range(N_CORES)), trace=_trace
    )
    out = finish_host([r["out5"] for r in res.results], binary_labels)
    if _trace:
        return out, res
    return out


# revision 48
# speedup vs baseline: 1.0468x; 1.0468x over previous
"""Trainium2 Bass kernel for nn_MaxMarginLoss (segment_reduce).

Data-parallel over the batch: 32 samples -> 8 NeuronCores x 4 samples.

Per core, for each sample b:
  - segment sums over T=2048 timesteps into S=32 step buckets are computed
    on TensorE as mask[128t,32s].T @ |x|[128t,1024d], accumulated over 16
    K-chunks into PSUM (this is the memory-bound part: 32 MiB of `inputs`
    per core, streamed as 2 MiB contiguous DMAs).
  - the appearance-order logic avoids any sort: first-appearance positions
    come from a masked min-reduce; each step's rank is the count of
    strictly-smaller packed keys (pos*33 + id); the ordered-adjacency
    matrix A[i,j] = (rank_j == rank_i + 1 and j present) turns "gather by
    argsort and diff neighbours" into a tiny 32x32 matmul H_next = A @ H.
  - pair energies E_i = mean_d relu(H_i - H_next_i)^2 via Relu + Square
    with fused free-dim accumulation.
Each core returns [4,5] per-sample sums (npairs, n, ninv, sum E*valid,
sum relu(1-E)*inv); the host applies the binary labels and the final
scalar division (a few hundred flops).
"""

import numpy as np

import concourse.bass as bass
from concourse import mybir
from concourse.bass_utils import run_bass_kernel_spmd
from concourse.tile import TileContext
from concourse.vector_clock import ScopedClock

F32 = mybir.dt.float32
BF16 = mybir.dt.bfloat16
U32 = mybir.dt.uint32
U16 = mybir.dt.uint16
I8 = mybir.dt.int8
I16 = mybir.dt.int16
OP = mybir.AluOpType
AF = mybir.ActivationFunctionType

B, T, D = 32, 2048, 1024
S = 32          # step ids 1..32; id 0 is padding
ALPHA = 1.0
N_CORES = 8
BL = B // N_CORES           # samples per core
K = 128                     # matmul contraction tile (partitions)
NCHUNK = T // K             # 16 K-chunks per sample
XT = 2                      # K-chunks per x DMA ([128, XT, 1024] = 1 MiB)

# The public neuronxcc walrus (setupSyncWait in CoreV2/V3GenImpl) only
# supports a small number of embedded semaphore waits per instruction,
# while Tile's scheduler attaches one wait per required logical proc.
# After scheduling, hoist overflow waits onto same-engine no-ops placed
# immediately before the owning instruction: engine program order makes
# that semantically identical.
_MAX_WAITS_DEFAULT = 1
_MAX_WAITS_BY_OPCODE = {}


class _LeanTailTileContext(TileContext):
    """Tile's default kernel tail is drain -> barrier -> sem-clear ->
    barrier.  After the first all-engine barrier no engine can still be
    waiting on a kernel semaphore, so the clears need no cross-engine
    ordering and the second (~3-4 us) barrier can be dropped; each
    engine's stream still ends after its own clears, so re-execution
    sees zeroed semaphores."""

    def _drain_and_barrier(self, tick_clock, wait_clock):
        drain_inst = self.nc.sync.drain()
        wait_clock.add_sem_waits(
            drain_inst.ins, ScopedClock({None: tick_clock.global_clock})
        )
        self.nc.all_engine_barrier()
        assert self.sems is not None
        popped = self.nc._tile_sem_poison_stack.pop()
        assert popped is self._sem_poison
        self.nc.clear_and_free_semaphores(list(self.sems.allocated().values()))


def _split_sync_waits(nc: bass.Bass):
    for f in nc.m.functions:
        for bb in f.blocks:
            insts = list(bb.instructions)
            need = []  # (ins, overflow_waits)
            for ins in insts:
                si = getattr(ins, "sync_info", None)
                if si is None or not si.on_wait:
                    continue
                cap = _MAX_WAITS_BY_OPCODE.get(ins.opcode, _MAX_WAITS_DEFAULT)
                waits = list(si.on_wait)
                if len(waits) <= cap:
                    continue
                ins.sync_info = mybir.SyncInfo(
                    on_wait=waits[:cap], on_update=list(si.on_update)
                )
                need.append((ins, waits[cap:], cap))
            if not need:
                continue
            nop_for: dict[str, list] = {}
            for ins, overflow, cap in need:
                eng = nc.engines[ins.engine]
                nops = []
                for i in range(0, len(overflow), cap):
                    nop = eng.nop(hint="waitsplit", nofuse=True)
                    nop.ins.sync_info = mybir.SyncInfo(
                        on_wait=overflow[i:i + cap], on_update=[]
                    )
                    nops.append(nop.ins)
                nop_for[ins.name] = nops
            created = {n.name for nops in nop_for.values() for n in nops}
            # nop() appended the new instructions to the current bb; pull
            # them out of every block and splice before their owners.
            for bb2 in f.blocks:
                cur = [i for i in bb2.instructions if i.name not in created]
                out = []
                for ins in cur:
                    out.extend(nop_for.get(ins.name, ()))
                    out.append(ins)
                bb2.instructions = out


# column offsets inside the packed int8 index buffer "cst8"
C8_IDSBC = 0                  # [K, T]   step ids, row 32b+s = ids of sample b
C8_IDSREP = C8_IDSBC + T      # [K, BL*NCHUNK*S] mask-layout ids, 32x repeated
C8_IOTAT = C8_IDSREP + BL * NCHUNK * S   # [K, NCHUNK*S] tiled 1..32
CW8 = C8_IOTAT + NCHUNK * S
# column offsets inside the packed float32 constant buffer "cst32"
C_STEPS = 0                   # [K, 1]
C_LOWER = C_STEPS + 1         # [K, S] block [i > j]
C_ONES = C_LOWER + S          # [K, S] ones
C_BONES = C_ONES + S          # [K, BL] block-diagonal ones
CW32 = C_BONES + BL


def build_program() -> bass.Bass:
    nc = bass.Bass()

    x = nc.declare_dram_parameter("x", [BL, T, D], F32, isOutput=False)
    cst8 = nc.declare_dram_parameter("cst8", [K, CW8], I8, isOutput=False)
    tmt16 = nc.declare_dram_parameter("tmt16", [K, T], I16, isOutput=False)
    cst32 = nc.declare_dram_parameter("cst32", [K, CW32], F32, isOutput=False)
    out5 = nc.declare_dram_parameter("out5", [BL, 5], F32, isOutput=True)

    with _LeanTailTileContext(nc) as tc:
        with (
            tc.tile_pool(name="const", bufs=1) as cpool,
            tc.tile_pool(name="persist", bufs=1) as pp,
            tc.tile_pool(name="xin", bufs=10) as xin,
            tc.tile_pool(name="xabs", bufs=10) as xabs,
            tc.tile_pool(name="mk", bufs=2) as mkp,
            tc.tile_pool(name="ps_sums", bufs=2, space="PSUM") as ps_sums,
            tc.tile_pool(name="ps_misc", bufs=1, space="PSUM") as ps_misc,
        ):
            # ---- constants / index data, narrow dtypes, DMA'd via the
            #      (otherwise idle) SWDGE queue so the HWDGE rings start
            #      on x immediately --------------------------------------
            sb_cst8 = cpool.tile([K, CW8], I8)
            nc.gpsimd.dma_start(out=sb_cst8[:], in_=cst8[:])
            sb_tmt = cpool.tile([K, T], I16)
            nc.gpsimd.dma_start(out=sb_tmt[:], in_=tmt16[:])
            sb_cst32 = cpool.tile([K, CW32], F32)
            nc.gpsimd.dma_start(out=sb_cst32[:], in_=cst32[:])
            sb_idsbc = sb_cst8[:, C8_IDSBC:C8_IDSBC + T]
            sb_steps = sb_cst32[:, C_STEPS:C_STEPS + 1]
            sb_lower = sb_cst32[:, C_LOWER:C_LOWER + S]
            sb_ones = sb_cst32[:, C_ONES:C_ONES + S]
            sb_bones = sb_cst32[:, C_BONES:C_BONES + BL]

            # ---- phase A: masks / positions / ranks (all 4 samples
            #      stacked on partitions: row 32*b + s) ------------------
            maskf = pp.tile([K, T], F32)        # [s-stacked, t] 0/1 mask
            counts = pp.tile([K, 1], F32)
            nc.vector.tensor_scalar(
                maskf[:], sb_idsbc[:], sb_steps[:], None, OP.is_equal, OP.add,
                accum_out=counts[:],
            )
            tm = pp.tile([K, T], F32)           # mask * (t - T)
            nc.vector.tensor_tensor(tm[:], maskf[:], sb_tmt[:], OP.mult)
            posm = pp.tile([K, 1], F32)         # pos - T (present) else 0
            nc.vector.tensor_reduce(posm[:], tm[:], mybir.AxisListType.X, OP.min)

            cnt1 = pp.tile([K, 1], F32)
            nc.vector.tensor_scalar(cnt1[:], counts[:], 1.0, None, OP.max)
            recip = pp.tile([K, 1], F32)        # 1 / max(counts, 1)
            nc.vector.reciprocal(recip[:], cnt1[:])

            # distinct sort keys: (pos-T)*33 + (s+1); order == stable
            # argsort of pos with id tiebreak (present strictly first)
            key = pp.tile([K, 1], F32)
            nc.vector.tensor_scalar(
                key[:], posm[:], 33.0, sb_steps[:], OP.mult, OP.add
            )
            key_sq = pp.tile([K, S], F32)
            nc.vector.tensor_scalar(key_sq[:], sb_ones[:], key[:], None, OP.mult)
            key_t = pp.tile([K, S], F32)        # row i holds key_l along l
            nc.vector.transpose(key_t[:], key_sq[:])
            cmp = pp.tile([K, S], F32)
            rank = pp.tile([K, 1], F32)
            nc.vector.tensor_scalar(
                cmp[:], key_t[:], key[:], None, OP.is_lt, OP.add,
                accum_out=rank[:],
            )
            rankp1 = pp.tile([K, 1], F32)
            nc.vector.tensor_scalar(rankp1[:], rank[:], 1.0, None, OP.add)
            t999 = pp.tile([K, 1], F32)         # 999 for absent steps
            nc.vector.tensor_scalar(
                t999[:], posm[:], 0.0, 999.0, OP.is_ge, OP.mult
            )
            rankp = pp.tile([K, 1], F32)        # rank, pushed out if absent
            nc.vector.tensor_tensor(rankp[:], rank[:], t999[:], OP.add)

            v_t = pp.tile([K, 8], F32)          # per-step stats columns
            nc.vector.tensor_scalar(v_t[:, 1:2], posm[:], 0.0, None, OP.is_lt)

            rankp_sq = pp.tile([K, S], F32)
            nc.vector.tensor_scalar(rankp_sq[:], sb_ones[:], rankp[:], None, OP.mult)
            rankp_t = pp.tile([K, S], F32)
            nc.vector.transpose(rankp_t[:], rankp_sq[:])
            rankp1_sq = pp.tile([K, S], F32)
            nc.vector.tensor_scalar(rankp1_sq[:], sb_ones[:], rankp1[:], None, OP.mult)
            rankp1_t = pp.tile([K, S], F32)
            nc.vector.transpose(rankp1_t[:], rankp1_sq[:])

            # A[i,j] = (rankp_j == rank_i + 1); succ_i = sum_j A[i,j]
            a_m = pp.tile([K, S], F32)
            nc.vector.tensor_scalar(
                a_m[:], rankp_t[:], rankp1[:], None, OP.is_equal, OP.add,
                accum_out=v_t[:, 0:1],
            )
            # A^T (lhsT for the H_next matmul; 0/1 so bf16 is exact)
            a_t = pp.tile([K, S], BF16)
            nc.vector.tensor_scalar(
                a_t[:], rankp1_t[:], rankp[:], None, OP.is_equal
            )
            # inv_i = sum_j A[i,j] * [i > j]
            a_inv = pp.tile([K, S], F32)
            nc.vector.scalar_tensor_tensor(
                a_inv[:], rankp_t[:], rankp1[:], sb_lower[:],
                op0=OP.is_equal, op1=OP.mult, accum_out=v_t[:, 2:3],
            )

            # ---- phase B: segment sums via TensorE --------------------
            h_all = pp.tile([K, D], BF16)
            hn = ps_misc.tile([K, D], F32)      # 2 PSUM banks
            diff = pp.tile([K, D], F32)
            sq = pp.tile([K, D], F32)
            e_raw = pp.tile([K, 1], F32)
            ps_of = {}

            def sample_tail(b):
                ps_all = ps_of[b]
                # H, H_next and pair energies for sample b; emitted one
                # sample late (mid-stream of sample b+1) so these ops land
                # in each engine's in-order queue at a point where their
                # dependencies are already met — emitted eagerly they
                # head-of-line-block the abs stream and stall the DMAs
                bs = slice(b * S, (b + 1) * S)
                for h in range(2):
                    nc.vector.tensor_scalar(
                        h_all[bs, h * 512:(h + 1) * 512],
                        ps_all[bs, h * 512:(h + 1) * 512],
                        recip[bs], None, OP.mult,
                    )
                for h in range(2):
                    nc.tensor.matmul(
                        hn[bs, h * 512:(h + 1) * 512],
                        lhsT=a_t[bs, :],
                        rhs=h_all[bs, h * 512:(h + 1) * 512],
                        start=True, stop=True,
                        tile_position=(b * S, b * S),
                    )
                nc.vector.tensor_tensor(
                    diff[bs, :], h_all[bs, :], hn[bs, :], OP.subtract
                )
                # relu(d)^2 == max(d,0)*d, with the free-dim sum fused in
                nc.vector.scalar_tensor_tensor(
                    sq[bs, :], diff[bs, :], 0.0, diff[bs, :],
                    op0=OP.max, op1=OP.mult, accum_out=e_raw[bs, :],
                )

            for b in range(BL):
                # all 16 mask chunks of the sample in one compare against
                # host-replicated ids (layout matches the x DMA below)
                mk_all = mkp.tile([K, NCHUNK * S], BF16)
                nc.vector.tensor_tensor(
                    mk_all[:],
                    sb_cst8[:, C8_IDSREP + b * NCHUNK * S:
                            C8_IDSREP + (b + 1) * NCHUNK * S],
                    sb_cst8[:, C8_IOTAT:C8_IOTAT + NCHUNK * S],
                    OP.is_equal,
                )
                # fresh PSUM banks per sample: sample b+1 accumulates while
                # sample b's H-scale still reads its own banks (no WAR)
                ps_all = ps_sums.tile([K, D], F32, tag="ps")
                ps_of[b] = ps_all
                for tq in range(NCHUNK // XT):
                    if tq == (NCHUNK // XT) // 2 and b > 0:
                        sample_tail(b - 1)
                    ti = b * (NCHUNK // XT) + tq
                    xt = xin.tile([K, XT, D], F32)
                    # alternate the two HWDGE rings (sync / scalar) so x
                    # streaming is not serialized on a single ring.
                    # Partition p holds XT consecutive T-rows (16 KiB of
                    # contiguous DRAM per partition); any (partition, sub)
                    # <-> t bijection works for the contraction as long as
                    # the host-side ids layout matches.
                    dma_eng = nc.sync if ti % 2 == 0 else nc.scalar
                    dma_eng.dma_start(
                        out=xt[:],
                        in_=x[b, tq * XT * K:(tq + 1) * XT * K, :].rearrange(
                            "(p s) d -> p s d", p=K
                        ),
                    )
                    # |x| rounded to bf16: the PE runs bf16 at 1 cycle/row
                    # vs fp32's 4; the 2^-9 relative rounding on |x| washes
                    # out to ~1e-4 in the final loss (mask stays exact 0/1).
                    # Alternate engines: ACT computes Abs->bf16 directly; DVE
                    # rounds to bf16 (RNE, so |bf16(x)| == bf16(|x|)) then
                    # clears the sign bit in place in the 16-bit 4x mode.
                    xa = xabs.tile([K, XT, D], BF16)
                    if ti % 2 == 0:
                        nc.scalar.activation(xa[:], xt[:], AF.Abs)
                    else:
                        nc.vector.tensor_copy(xa[:], xt[:])
                        nc.vector.tensor_scalar(
                            xa[:].bitcast(U16), xa[:].bitcast(U16),
                            0x7FFF, None, OP.bitwise_and,
                        )
                    for sub in range(XT):
                        c = tq * XT + sub
                        for h in range(2):
                            nc.tensor.matmul(
                                ps_all[b * S:(b + 1) * S, h * 512:(h + 1) * 512],
                                lhsT=mk_all[:, c * S:(c + 1) * S],
                                rhs=xa[:, sub, h * 512:(h + 1) * 512],
                                start=(c == 0), stop=(c == NCHUNK - 1),
                                tile_position=(0, b * S),
                            )
            sample_tail(BL - 1)

            # ---- phase C: combine per-step stats ----------------------
            e_col = pp.tile([K, 1], F32)
            nc.vector.tensor_scalar(e_col[:], e_raw[:], 1.0 / D, None, OP.mult)
            nc.vector.tensor_tensor(v_t[:, 3:4], e_col[:], v_t[:, 0:1], OP.mult)
            ae1 = pp.tile([K, 1], F32)          # relu(ALPHA - E)
            nc.vector.tensor_scalar(
                ae1[:], e_col[:], -1.0, ALPHA, OP.mult, OP.add
            )
            ae = pp.tile([K, 1], F32)
            nc.vector.tensor_scalar(ae[:], ae1[:], 0.0, None, OP.max)
            nc.vector.tensor_tensor(v_t[:, 4:5], ae[:], v_t[:, 2:3], OP.mult)

            # per-sample column sums: blockones[128,4].T @ V[128,5] -> [4,5]
            vp = ps_misc.tile([BL, 8], F32)
            nc.tensor.matmul(
                vp[:, 0:5], lhsT=sb_bones[:], rhs=v_t[:, 0:5],
                start=True, stop=True,
            )
            out_sb = pp.tile([BL, 5], F32)
            nc.vector.tensor_copy(out_sb[:], vp[:, 0:5])
            nc.sync.dma_start(out=out5[:], in_=out_sb[:])

    _split_sync_waits(nc)
    return nc


_PROGRAM: bass.Bass | None = None


def get_program() -> bass.Bass:
    global _PROGRAM
    if _PROGRAM is None:
        _PROGRAM = build_program()
    return _PROGRAM


def make_in_maps(inputs: np.ndarray, step_ids: np.ndarray) -> list[dict]:
    """Shard + pre-layout the (tiny) index tensors per core."""
    inputs = np.ascontiguousarray(np.asarray(inputs, dtype=np.float32))
    step_ids = np.asarray(step_ids)

    tmt16 = np.tile(
        (np.arange(T) - T).astype(np.int16)[None, :], (K, 1)
    )
    iota_t = np.tile(
        np.tile(np.arange(1, S + 1, dtype=np.int8), NCHUNK)[None, :], (K, 1)
    )
    cst32 = np.empty((K, CW32), dtype=np.float32)
    cst32[:, C_STEPS:C_STEPS + 1] = np.tile(
        np.arange(1, S + 1, dtype=np.float32), BL
    )[:, None]
    cst32[:, C_LOWER:C_LOWER + S] = np.tile(
        (np.arange(S)[:, None] > np.arange(S)[None, :]).astype(np.float32),
        (BL, 1),
    )
    cst32[:, C_ONES:C_ONES + S] = 1.0
    cst32[:, C_BONES:C_BONES + BL] = (
        (np.arange(K)[:, None] // S) == np.arange(BL)[None, :]
    ).astype(np.float32)

    in_maps = []
    for core in range(N_CORES):
        b0 = core * BL
        ids = step_ids[b0:b0 + BL].astype(np.int8)              # [4, 2048]
        # matmul chunk (b, tq, sub) contracts t = tq*XT*K + p*XT + sub on
        # partition p; idsrep repeats each id S times along the free dim so
        # one is_equal against iota_t yields all NCHUNK mask chunks
        idsrep = np.repeat(
            ids.reshape(BL, NCHUNK // XT, K, XT).transpose(2, 0, 1, 3)
            .reshape(K, BL, NCHUNK),
            S, axis=2,
        ).reshape(K, BL * NCHUNK * S)
        cst8 = np.empty((K, CW8), dtype=np.int8)
        cst8[:, C8_IDSBC:C8_IDSBC + T] = np.repeat(ids, S, axis=0)
        cst8[:, C8_IDSREP:C8_IDSREP + BL * NCHUNK * S] = idsrep
        cst8[:, C8_IOTAT:C8_IOTAT + NCHUNK * S] = iota_t
        in_maps.append({
            "x": inputs[b0:b0 + BL],
            "cst8": cst8,
            "tmt16": tmt16,
            "cst32": cst32,
        })
    return in_maps


def finish_host(out5_per_core: list[np.ndarray], binary_labels: np.ndarray):
    """Combine per-sample (npairs, n, ninv, S1, S2) with labels."""
    v = np.concatenate([np.asarray(o, np.float64) for o in out5_per_core], axis=0)
    npairs, n, ninv, s1, s2 = v[:, 0], v[:, 1], v[:, 2], v[:, 3], v[:, 4]
    labels = np.asarray(binary_labels)
    loss_pos = s1 / np.maximum(npairs, 1.0)
    loss_neg = s2 / np.maximum(ninv, 1.0)
    pos_count = (labels == 1) & (n >= 2)
    neg_count = (labels == 0) & (ninv > 0)
    total = (loss_pos * pos_count).sum() + (loss_neg * neg_count).sum()
    num = pos_count.sum() + neg_count.sum()
    return np.float32(total / (num + 1e-9))


def kernel(inputs, step_ids, binary_labels, _trace=False):
    nc = get_program()
    in_maps = make_in_maps(inputs, step_ids)
    res = run_bass_kernel_spmd(
        nc, in_maps, core_ids=list(range(N_CORES)), trace=_trace
    )
    out = finish_host([r["out5"] for r in res.results], binary_labels)
    if _trace:
        return out, res
    return out


# revision 51
# speedup vs baseline: 1.0517x; 1.0047x over previous
"""Trainium2 Bass kernel for nn_MaxMarginLoss (segment_reduce).

Data-parallel over the batch: 32 samples -> 8 NeuronCores x 4 samples.

Per core, for each sample b:
  - segment sums over T=2048 timesteps into S=32 step buckets are computed
    on TensorE as mask[128t,32s].T @ |x|[128t,1024d], accumulated over 16
    K-chunks into PSUM (this is the memory-bound part: 32 MiB of `inputs`
    per core, streamed as 2 MiB contiguous DMAs).
  - the appearance-order logic avoids any sort: first-appearance positions
    come from a masked min-reduce; each step's rank is the count of
    strictly-smaller packed keys (pos*33 + id); the ordered-adjacency
    matrix A[i,j] = (rank_j == rank_i + 1 and j present) turns "gather by
    argsort and diff neighbours" into a tiny 32x32 matmul H_next = A @ H.
  - pair energies E_i = mean_d relu(H_i - H_next_i)^2 via Relu + Square
    with fused free-dim accumulation.
Each core returns [4,5] per-sample sums (npairs, n, ninv, sum E*valid,
sum relu(1-E)*inv); the host applies the binary labels and the final
scalar division (a few hundred flops).
"""

import numpy as np

import concourse.bass as bass
from concourse import mybir
from concourse.bass_utils import run_bass_kernel_spmd
from concourse.tile import TileContext
from concourse.vector_clock import ScopedClock

F32 = mybir.dt.float32
BF16 = mybir.dt.bfloat16
U32 = mybir.dt.uint32
U16 = mybir.dt.uint16
I8 = mybir.dt.int8
I16 = mybir.dt.int16
OP = mybir.AluOpType
AF = mybir.ActivationFunctionType

B, T, D = 32, 2048, 1024
S = 32          # step ids 1..32; id 0 is padding
ALPHA = 1.0
N_CORES = 8
BL = B // N_CORES           # samples per core
K = 128                     # matmul contraction tile (partitions)
NCHUNK = T // K             # 16 K-chunks per sample
XT = 2                      # K-chunks per x DMA ([128, XT, 1024] = 1 MiB)

# The public neuronxcc walrus (setupSyncWait in CoreV2/V3GenImpl) only
# supports a small number of embedded semaphore waits per instruction,
# while Tile's scheduler attaches one wait per required logical proc.
# After scheduling, hoist overflow waits onto same-engine no-ops placed
# immediately before the owning instruction: engine program order makes
# that semantically identical.
_MAX_WAITS_DEFAULT = 1
_MAX_WAITS_BY_OPCODE = {}


class _LeanTailTileContext(TileContext):
    """Tile's default kernel tail is drain -> barrier -> sem-clear ->
    barrier.  After the first all-engine barrier no engine can still be
    waiting on a kernel semaphore, so the clears need no cross-engine
    ordering and the second (~3-4 us) barrier can be dropped; each
    engine's stream still ends after its own clears, so re-execution
    sees zeroed semaphores."""

    def _drain_and_barrier(self, tick_clock, wait_clock):
        drain_inst = self.nc.sync.drain()
        wait_clock.add_sem_waits(
            drain_inst.ins, ScopedClock({None: tick_clock.global_clock})
        )
        self.nc.all_engine_barrier()
        assert self.sems is not None
        popped = self.nc._tile_sem_poison_stack.pop()
        assert popped is self._sem_poison
        self.nc.clear_and_free_semaphores(list(self.sems.allocated().values()))


def _split_sync_waits(nc: bass.Bass):
    for f in nc.m.functions:
        for bb in f.blocks:
            insts = list(bb.instructions)
            need = []  # (ins, overflow_waits)
            for ins in insts:
                si = getattr(ins, "sync_info", None)
                if si is None or not si.on_wait:
                    continue
                cap = _MAX_WAITS_BY_OPCODE.get(ins.opcode, _MAX_WAITS_DEFAULT)
                waits = list(si.on_wait)
                if len(waits) <= cap:
                    continue
                ins.sync_info = mybir.SyncInfo(
                    on_wait=waits[:cap], on_update=list(si.on_update)
                )
                need.append((ins, waits[cap:], cap))
            if not need:
                continue
            nop_for: dict[str, list] = {}
            for ins, overflow, cap in need:
                eng = nc.engines[ins.engine]
                nops = []
                for i in range(0, len(overflow), cap):
                    nop = eng.nop(hint="waitsplit", nofuse=True)
                    nop.ins.sync_info = mybir.SyncInfo(
                        on_wait=overflow[i:i + cap], on_update=[]
                    )
                    nops.append(nop.ins)
                nop_for[ins.name] = nops
            created = {n.name for nops in nop_for.values() for n in nops}
            # nop() appended the new instructions to the current bb; pull
            # them out of every block and splice before their owners.
            for bb2 in f.blocks:
                cur = [i for i in bb2.instructions if i.name not in created]
                out = []
                for ins in cur:
                    out.extend(nop_for.get(ins.name, ()))
                    out.append(ins)
                bb2.instructions = out


# column offsets inside the packed int8 index buffer "cst8"
C8_IDSBC = 0                  # [K, T]   step ids, row 32b+s = ids of sample b
C8_IDSREP = C8_IDSBC + T      # [K, BL*NCHUNK*S] mask-layout ids, 32x repeated
C8_IOTAT = C8_IDSREP + BL * NCHUNK * S   # [K, NCHUNK*S] tiled 1..32
CW8 = C8_IOTAT + NCHUNK * S
# column offsets inside the packed float32 constant buffer "cst32"
C_STEPS = 0                   # [K, 1]
C_LOWER = C_STEPS + 1         # [K, S] block [i > j]
C_ONES = C_LOWER + S          # [K, S] ones
C_BONES = C_ONES + S          # [K, BL] block-diagonal ones
CW32 = C_BONES + BL


def build_program() -> bass.Bass:
    nc = bass.Bass()

    x = nc.declare_dram_parameter("x", [BL, T, D], F32, isOutput=False)
    cst8 = nc.declare_dram_parameter("cst8", [K, CW8], I8, isOutput=False)
    tmt16 = nc.declare_dram_parameter("tmt16", [K, T], I16, isOutput=False)
    cst32 = nc.declare_dram_parameter("cst32", [K, CW32], F32, isOutput=False)
    out5 = nc.declare_dram_parameter("out5", [BL, 5], F32, isOutput=True)

    with _LeanTailTileContext(nc) as tc:
        with (
            tc.tile_pool(name="const", bufs=1) as cpool,
            tc.tile_pool(name="persist", bufs=1) as pp,
            tc.tile_pool(name="xin", bufs=10) as xin,
            tc.tile_pool(name="xabs", bufs=10) as xabs,
            tc.tile_pool(name="mk", bufs=2) as mkp,
            tc.tile_pool(name="ps_sums", bufs=2, space="PSUM") as ps_sums,
            tc.tile_pool(name="ps_misc", bufs=1, space="PSUM") as ps_misc,
        ):
            # ---- constants / index data, narrow dtypes, DMA'd via the
            #      (otherwise idle) SWDGE queue so the HWDGE rings start
            #      on x immediately --------------------------------------
            sb_cst8 = cpool.tile([K, CW8], I8)
            nc.gpsimd.dma_start(out=sb_cst8[:], in_=cst8[:])
            sb_tmt = cpool.tile([K, T], I16)
            nc.gpsimd.dma_start(out=sb_tmt[:], in_=tmt16[:])
            sb_cst32 = cpool.tile([K, CW32], F32)
            nc.gpsimd.dma_start(out=sb_cst32[:], in_=cst32[:])
            sb_idsbc = sb_cst8[:, C8_IDSBC:C8_IDSBC + T]
            sb_steps = sb_cst32[:, C_STEPS:C_STEPS + 1]
            sb_lower = sb_cst32[:, C_LOWER:C_LOWER + S]
            sb_ones = sb_cst32[:, C_ONES:C_ONES + S]
            sb_bones = sb_cst32[:, C_BONES:C_BONES + BL]

            # ---- phase A: masks / positions / ranks (all 4 samples
            #      stacked on partitions: row 32*b + s) ------------------
            maskf = pp.tile([K, T], F32)        # [s-stacked, t] 0/1 mask
            counts = pp.tile([K, 1], F32)
            nc.vector.tensor_scalar(
                maskf[:], sb_idsbc[:], sb_steps[:], None, OP.is_equal, OP.add,
                accum_out=counts[:],
            )
            tm = pp.tile([K, T], F32)           # mask * (t - T)
            nc.vector.tensor_tensor(tm[:], maskf[:], sb_tmt[:], OP.mult)
            posm = pp.tile([K, 1], F32)         # pos - T (present) else 0
            nc.vector.tensor_reduce(posm[:], tm[:], mybir.AxisListType.X, OP.min)

            cnt1 = pp.tile([K, 1], F32)
            nc.vector.tensor_scalar(cnt1[:], counts[:], 1.0, None, OP.max)
            recip = pp.tile([K, 1], F32)        # 1 / max(counts, 1)
            nc.vector.reciprocal(recip[:], cnt1[:])

            # distinct sort keys: (pos-T)*33 + (s+1); order == stable
            # argsort of pos with id tiebreak (present strictly first)
            key = pp.tile([K, 1], F32)
            nc.vector.tensor_scalar(
                key[:], posm[:], 33.0, sb_steps[:], OP.mult, OP.add
            )
            key_sq = pp.tile([K, S], F32)
            nc.vector.tensor_scalar(key_sq[:], sb_ones[:], key[:], None, OP.mult)
            key_t = pp.tile([K, S], F32)        # row i holds key_l along l
            nc.vector.transpose(key_t[:], key_sq[:])
            cmp = pp.tile([K, S], F32)
            rank = pp.tile([K, 1], F32)
            nc.vector.tensor_scalar(
                cmp[:], key_t[:], key[:], None, OP.is_lt, OP.add,
                accum_out=rank[:],
            )
            rankp1 = pp.tile([K, 1], F32)
            nc.vector.tensor_scalar(rankp1[:], rank[:], 1.0, None, OP.add)
            t999 = pp.tile([K, 1], F32)         # 999 for absent steps
            nc.vector.tensor_scalar(
                t999[:], posm[:], 0.0, 999.0, OP.is_ge, OP.mult
            )
            rankp = pp.tile([K, 1], F32)        # rank, pushed out if absent
            nc.vector.tensor_tensor(rankp[:], rank[:], t999[:], OP.add)

            v_t = pp.tile([K, 8], F32)          # per-step stats columns
            nc.vector.tensor_scalar(v_t[:, 1:2], posm[:], 0.0, None, OP.is_lt)

            rankp_sq = pp.tile([K, S], F32)
            nc.vector.tensor_scalar(rankp_sq[:], sb_ones[:], rankp[:], None, OP.mult)
            rankp_t = pp.tile([K, S], F32)
            nc.vector.transpose(rankp_t[:], rankp_sq[:])
            rankp1_sq = pp.tile([K, S], F32)
            nc.vector.tensor_scalar(rankp1_sq[:], sb_ones[:], rankp1[:], None, OP.mult)
            rankp1_t = pp.tile([K, S], F32)
            nc.vector.transpose(rankp1_t[:], rankp1_sq[:])

            # A[i,j] = (rankp_j == rank_i + 1); succ_i = sum_j A[i,j]
            a_m = pp.tile([K, S], F32)
            nc.vector.tensor_scalar(
                a_m[:], rankp_t[:], rankp1[:], None, OP.is_equal, OP.add,
                accum_out=v_t[:, 0:1],
            )
            # A^T (lhsT for the H_next matmul; 0/1 so bf16 is exact)
            a_t = pp.tile([K, S], BF16)
            nc.vector.tensor_scalar(
                a_t[:], rankp1_t[:], rankp[:], None, OP.is_equal
            )
            # inv_i = sum_j A[i,j] * [i > j]
            a_inv = pp.tile([K, S], F32)
            nc.vector.scalar_tensor_tensor(
                a_inv[:], rankp_t[:], rankp1[:], sb_lower[:],
                op0=OP.is_equal, op1=OP.mult, accum_out=v_t[:, 2:3],
            )

            # ---- phase B: segment sums via TensorE --------------------
            h_all = pp.tile([K, D], BF16)
            hn = ps_misc.tile([K, D], F32)      # 2 PSUM banks
            diff = pp.tile([K, D], F32)
            sq = pp.tile([K, D], F32)
            e_raw = pp.tile([K, 1], F32)
            ps_of = {}

            # Per-sample tail, emitted one sample late (during sample b+1's
            # stream) so the ops land in each engine's in-order queue at a
            # point where their dependencies are already met — emitted
            # eagerly they head-of-line-block the abs stream and stall the
            # DMAs.  The H-scale half runs as early as possible because it
            # releases sample b's PSUM banks for sample b+2.
            def sample_scale(b):
                ps_all = ps_of[b]
                bs = slice(b * S, (b + 1) * S)
                for h in range(2):
                    nc.vector.tensor_scalar(
                        h_all[bs, h * 512:(h + 1) * 512],
                        ps_all[bs, h * 512:(h + 1) * 512],
                        recip[bs], None, OP.mult,
                    )

            def sample_tail(b):
                bs = slice(b * S, (b + 1) * S)
                for h in range(2):
                    nc.tensor.matmul(
                        hn[bs, h * 512:(h + 1) * 512],
                        lhsT=a_t[bs, :],
                        rhs=h_all[bs, h * 512:(h + 1) * 512],
                        start=True, stop=True,
                        tile_position=(b * S, b * S),
                    )
                nc.vector.tensor_tensor(
                    diff[bs, :], h_all[bs, :], hn[bs, :], OP.subtract
                )
                # relu(d)^2 == max(d,0)*d, with the free-dim sum fused in
                nc.vector.scalar_tensor_tensor(
                    sq[bs, :], diff[bs, :], 0.0, diff[bs, :],
                    op0=OP.max, op1=OP.mult, accum_out=e_raw[bs, :],
                )

            for b in range(BL):
                # all 16 mask chunks of the sample in one compare against
                # host-replicated ids (layout matches the x DMA below)
                mk_all = mkp.tile([K, NCHUNK * S], BF16)
                nc.vector.tensor_tensor(
                    mk_all[:],
                    sb_cst8[:, C8_IDSREP + b * NCHUNK * S:
                            C8_IDSREP + (b + 1) * NCHUNK * S],
                    sb_cst8[:, C8_IOTAT:C8_IOTAT + NCHUNK * S],
                    OP.is_equal,
                )
                # fresh PSUM banks per sample: sample b+1 accumulates while
                # sample b's H-scale still reads its own banks (no WAR)
                ps_all = ps_sums.tile([K, D], F32, tag="ps")
                ps_of[b] = ps_all
                for tq in range(NCHUNK // XT):
                    if tq == 1 and b > 0:
                        sample_scale(b - 1)
                    if tq == (NCHUNK // XT) // 2 and b > 0:
                        sample_tail(b - 1)
                    ti = b * (NCHUNK // XT) + tq
                    xt = xin.tile([K, XT, D], F32)
                    # alternate the two HWDGE rings (sync / scalar) so x
                    # streaming is not serialized on a single ring.
                    # Partition p holds XT consecutive T-rows (16 KiB of
                    # contiguous DRAM per partition); any (partition, sub)
                    # <-> t bijection works for the contraction as long as
                    # the host-side ids layout matches.
                    dma_eng = nc.sync if ti % 2 == 0 else nc.scalar
                    dma_eng.dma_start(
                        out=xt[:],
                        in_=x[b, tq * XT * K:(tq + 1) * XT * K, :].rearrange(
                            "(p s) d -> p s d", p=K
                        ),
                    )
                    # |x| rounded to bf16: the PE runs bf16 at 1 cycle/row
                    # vs fp32's 4; the 2^-9 relative rounding on |x| washes
                    # out to ~1e-4 in the final loss (mask stays exact 0/1).
                    # Alternate engines: ACT computes Abs->bf16 directly; DVE
                    # rounds to bf16 (RNE, so |bf16(x)| == bf16(|x|)) then
                    # clears the sign bit in place in the 16-bit 4x mode.
                    xa = xabs.tile([K, XT, D], BF16)
                    if ti % 2 == 0:
                        nc.scalar.activation(xa[:], xt[:], AF.Abs)
                    else:
                        nc.vector.tensor_copy(xa[:], xt[:])
                        nc.vector.tensor_scalar(
                            xa[:].bitcast(U16), xa[:].bitcast(U16),
                            0x7FFF, None, OP.bitwise_and,
                        )
                    for sub in range(XT):
                        c = tq * XT + sub
                        for h in range(2):
                            nc.tensor.matmul(
                                ps_all[b * S:(b + 1) * S, h * 512:(h + 1) * 512],
                                lhsT=mk_all[:, c * S:(c + 1) * S],
                                rhs=xa[:, sub, h * 512:(h + 1) * 512],
                                start=(c == 0), stop=(c == NCHUNK - 1),
                                tile_position=(0, b * S),
                            )
            sample_scale(BL - 1)
            sample_tail(BL - 1)

            # ---- phase C: combine per-step stats ----------------------
            e_col = pp.tile([K, 1], F32)
            nc.vector.tensor_scalar(e_col[:], e_raw[:], 1.0 / D, None, OP.mult)
            nc.vector.tensor_tensor(v_t[:, 3:4], e_col[:], v_t[:, 0:1], OP.mult)
            ae1 = pp.tile([K, 1], F32)          # relu(ALPHA - E)
            nc.vector.tensor_scalar(
                ae1[:], e_col[:], -1.0, ALPHA, OP.mult, OP.add
            )
            ae = pp.tile([K, 1], F32)
            nc.vector.tensor_scalar(ae[:], ae1[:], 0.0, None, OP.max)
            nc.vector.tensor_tensor(v_t[:, 4:5], ae[:], v_t[:, 2:3], OP.mult)

            # per-sample column sums: blockones[128,4].T @ V[128,5] -> [4,5]
            vp = ps_misc.tile([BL, 8], F32)
            nc.tensor.matmul(
                vp[:, 0:5], lhsT=sb_bones[:], rhs=v_t[:, 0:5],
                start=True, stop=True,
            )
            out_sb = pp.tile([BL, 5], F32)
            nc.vector.tensor_copy(out_sb[:], vp[:, 0:5])
            nc.sync.dma_start(out=out5[:], in_=out_sb[:])

    _split_sync_waits(nc)
    return nc


_PROGRAM: bass.Bass | None = None


def get_program() -> bass.Bass:
    global _PROGRAM
    if _PROGRAM is None:
        _PROGRAM = build_program()
    return _PROGRAM


def make_in_maps(inputs: np.ndarray, step_ids: np.ndarray) -> list[dict]:
    """Shard + pre-layout the (tiny) index tensors per core."""
    inputs = np.ascontiguousarray(np.asarray(inputs, dtype=np.float32))
    step_ids = np.asarray(step_ids)

    tmt16 = np.tile(
        (np.arange(T) - T).astype(np.int16)[None, :], (K, 1)
    )
    iota_t = np.tile(
        np.tile(np.arange(1, S + 1, dtype=np.int8), NCHUNK)[None, :], (K, 1)
    )
    cst32 = np.empty((K, CW32), dtype=np.float32)
    cst32[:, C_STEPS:C_STEPS + 1] = np.tile(
        np.arange(1, S + 1, dtype=np.float32), BL
    )[:, None]
    cst32[:, C_LOWER:C_LOWER + S] = np.tile(
        (np.arange(S)[:, None] > np.arange(S)[None, :]).astype(np.float32),
        (BL, 1),
    )
    cst32[:, C_ONES:C_ONES + S] = 1.0
    cst32[:, C_BONES:C_BONES + BL] = (
        (np.arange(K)[:, None] // S) == np.arange(BL)[None, :]
    ).astype(np.float32)

    in_maps = []
    for core in range(N_CORES):
        b0 = core * BL
        ids = step_ids[b0:b0 + BL].astype(np.int8)              # [4, 2048]
        # matmul chunk (b, tq, sub) contracts t = tq*XT*K + p*XT + sub on
        # partition p; idsrep repeats each id S times along the free dim so
        # one is_equal against iota_t yields all NCHUNK mask chunks
        idsrep = np.repeat(
            ids.reshape(BL, NCHUNK // XT, K, XT).transpose(2, 0, 1, 3)
            .reshape(K, BL, NCHUNK),
            S, axis=2,
        ).reshape(K, BL * NCHUNK * S)
        cst8 = np.empty((K, CW8), dtype=np.int8)
        cst8[:, C8_IDSBC:C8_IDSBC + T] = np.repeat(ids, S, axis=0)
        cst8[:, C8_IDSREP:C8_IDSREP + BL * NCHUNK * S] = idsrep
        cst8[:, C8_IOTAT:C8_IOTAT + NCHUNK * S] = iota_t
        in_maps.append({
            "x": inputs[b0:b0 + BL],
            "cst8": cst8,
            "tmt16": tmt16,
            "cst32": cst32,
        })
    return in_maps


def finish_host(out5_per_core: list[np.ndarray], binary_labels: np.ndarray):
    """Combine per-sample (npairs, n, ninv, S1, S2) with labels."""
    v = np.concatenate([np.asarray(o, np.float64) for o in out5_per_core], axis=0)
    npairs, n, ninv, s1, s2 = v[:, 0], v[:, 1], v[:, 2], v[:, 3], v[:, 4]
    labels = np.asarray(binary_labels)
    loss_pos = s1 / np.maximum(npairs, 1.0)
    loss_neg = s2 / np.maximum(ninv, 1.0)
    pos_count = (labels == 1) & (n >= 2)
    neg_count = (labels == 0) & (ninv > 0)
    total = (loss_pos * pos_count).sum() + (loss_neg * neg_count).sum()
    num = pos_count.sum() + neg_count.sum()
    return np.float32(total / (num + 1e-9))


def kernel(inputs, step_ids, binary_labels, _trace=False):
    nc = get_program()
    in_maps = make_in_maps(inputs, step_ids)
    res = run_bass_kernel_spmd(
        nc, in_maps, core_ids=list(range(N_CORES)), trace=_trace
    )
    out = finish_host([r["out5"] for r in res.results], binary_labels)
    if _trace:
        return out, res
    return out


# revision 52
# speedup vs baseline: 1.1064x; 1.0520x over previous
"""Trainium2 Bass kernel for nn_MaxMarginLoss (segment_reduce).

Data-parallel over the batch: 32 samples -> 8 NeuronCores x 4 samples.

Per core, for each sample b:
  - segment sums over T=2048 timesteps into S=32 step buckets are computed
    on TensorE as mask[128t,32s].T @ |x|[128t,1024d], accumulated over 16
    K-chunks into PSUM (this is the memory-bound part: 32 MiB of `inputs`
    per core, streamed as 2 MiB contiguous DMAs).
  - the appearance-order logic avoids any sort: first-appearance positions
    come from a masked min-reduce; each step's rank is the count of
    strictly-smaller packed keys (pos*33 + id); the ordered-adjacency
    matrix A[i,j] = (rank_j == rank_i + 1 and j present) turns "gather by
    argsort and diff neighbours" into a tiny 32x32 matmul H_next = A @ H.
  - pair energies E_i = mean_d relu(H_i - H_next_i)^2 via Relu + Square
    with fused free-dim accumulation.
Each core returns [4,5] per-sample sums (npairs, n, ninv, sum E*valid,
sum relu(1-E)*inv); the host applies the binary labels and the final
scalar division (a few hundred flops).
"""

import numpy as np

import concourse.bass as bass
from concourse import mybir
from concourse.bass_utils import run_bass_kernel_spmd
from concourse.tile import TileContext
from concourse.vector_clock import ScopedClock

F32 = mybir.dt.float32
BF16 = mybir.dt.bfloat16
U32 = mybir.dt.uint32
U16 = mybir.dt.uint16
I8 = mybir.dt.int8
I16 = mybir.dt.int16
OP = mybir.AluOpType
AF = mybir.ActivationFunctionType

B, T, D = 32, 2048, 1024
S = 32          # step ids 1..32; id 0 is padding
ALPHA = 1.0
N_CORES = 8
BL = B // N_CORES           # samples per core
K = 128                     # matmul contraction tile (partitions)
NCHUNK = T // K             # 16 K-chunks per sample
XT = 2                      # K-chunks per x DMA ([128, XT, 1024] = 1 MiB)

# The public neuronxcc walrus (setupSyncWait in CoreV2/V3GenImpl) only
# supports a small number of embedded semaphore waits per instruction,
# while Tile's scheduler attaches one wait per required logical proc.
# After scheduling, hoist overflow waits onto same-engine no-ops placed
# immediately before the owning instruction: engine program order makes
# that semantically identical.
_MAX_WAITS_DEFAULT = 1
_MAX_WAITS_BY_OPCODE = {}


class _LeanTailTileContext(TileContext):
    """Tile's default kernel tail is drain -> barrier -> sem-clear ->
    barrier.  After the first all-engine barrier no engine can still be
    waiting on a kernel semaphore, so the clears need no cross-engine
    ordering and the second (~3-4 us) barrier can be dropped; each
    engine's stream still ends after its own clears, so re-execution
    sees zeroed semaphores."""

    def _drain_and_barrier(self, tick_clock, wait_clock):
        drain_inst = self.nc.sync.drain()
        wait_clock.add_sem_waits(
            drain_inst.ins, ScopedClock({None: tick_clock.global_clock})
        )
        self.nc.all_engine_barrier()
        assert self.sems is not None
        popped = self.nc._tile_sem_poison_stack.pop()
        assert popped is self._sem_poison
        self.nc.clear_and_free_semaphores(list(self.sems.allocated().values()))


def _split_sync_waits(nc: bass.Bass):
    for f in nc.m.functions:
        for bb in f.blocks:
            insts = list(bb.instructions)
            need = []  # (ins, overflow_waits)
            for ins in insts:
                si = getattr(ins, "sync_info", None)
                if si is None or not si.on_wait:
                    continue
                cap = _MAX_WAITS_BY_OPCODE.get(ins.opcode, _MAX_WAITS_DEFAULT)
                waits = list(si.on_wait)
                if len(waits) <= cap:
                    continue
                ins.sync_info = mybir.SyncInfo(
                    on_wait=waits[:cap], on_update=list(si.on_update)
                )
                need.append((ins, waits[cap:], cap))
            if not need:
                continue
            nop_for: dict[str, list] = {}
            for ins, overflow, cap in need:
                eng = nc.engines[ins.engine]
                nops = []
                for i in range(0, len(overflow), cap):
                    nop = eng.nop(hint="waitsplit", nofuse=True)
                    nop.ins.sync_info = mybir.SyncInfo(
                        on_wait=overflow[i:i + cap], on_update=[]
                    )
                    nops.append(nop.ins)
                nop_for[ins.name] = nops
            created = {n.name for nops in nop_for.values() for n in nops}
            # nop() appended the new instructions to the current bb; pull
            # them out of every block and splice before their owners.
            for bb2 in f.blocks:
                cur = [i for i in bb2.instructions if i.name not in created]
                out = []
                for ins in cur:
                    out.extend(nop_for.get(ins.name, ()))
                    out.append(ins)
                bb2.instructions = out


# column offsets inside the packed int8 index buffer "cst8"
C8_IDSBC = 0                  # [K, T]   step ids, row 32b+s = ids of sample b
C8_IDSREP = C8_IDSBC + T      # [K, BL*NCHUNK*S] mask-layout ids, 32x repeated
C8_IOTAT = C8_IDSREP + BL * NCHUNK * S   # [K, NCHUNK*S] tiled 1..32
CW8 = C8_IOTAT + NCHUNK * S
# column offsets inside the packed float32 constant buffer "cst32"
C_STEPS = 0                   # [K, 1]
C_LOWER = C_STEPS + 1         # [K, S] block [i > j]
C_ONES = C_LOWER + S          # [K, S] ones
C_BONES = C_ONES + S          # [K, BL] block-diagonal ones
CW32 = C_BONES + BL


def build_program() -> bass.Bass:
    nc = bass.Bass()

    x = nc.declare_dram_parameter("x", [BL, T, D], F32, isOutput=False)
    cst8 = nc.declare_dram_parameter("cst8", [K, CW8], I8, isOutput=False)
    tmt16 = nc.declare_dram_parameter("tmt16", [K, T], I16, isOutput=False)
    cst32 = nc.declare_dram_parameter("cst32", [K, CW32], F32, isOutput=False)
    out5 = nc.declare_dram_parameter("out5", [BL, 5], F32, isOutput=True)

    with _LeanTailTileContext(nc) as tc:
        with (
            tc.tile_pool(name="const", bufs=1) as cpool,
            tc.tile_pool(name="persist", bufs=1) as pp,
            tc.tile_pool(name="xin", bufs=10) as xin,
            tc.tile_pool(name="xabs", bufs=10) as xabs,
            tc.tile_pool(name="mk", bufs=2) as mkp,
            tc.tile_pool(name="ps_sums", bufs=2, space="PSUM") as ps_sums,
            tc.tile_pool(name="ps_misc", bufs=1, space="PSUM") as ps_misc,
        ):
            # ---- constants / index data, narrow dtypes, DMA'd via the
            #      (otherwise idle) SWDGE queue so the HWDGE rings start
            #      on x immediately --------------------------------------
            sb_cst8 = cpool.tile([K, CW8], I8)
            nc.gpsimd.dma_start(out=sb_cst8[:], in_=cst8[:])
            sb_tmt = cpool.tile([K, T], I16)
            nc.gpsimd.dma_start(out=sb_tmt[:], in_=tmt16[:])
            sb_cst32 = cpool.tile([K, CW32], F32)
            nc.gpsimd.dma_start(out=sb_cst32[:], in_=cst32[:])
            sb_idsbc = sb_cst8[:, C8_IDSBC:C8_IDSBC + T]
            sb_steps = sb_cst32[:, C_STEPS:C_STEPS + 1]
            sb_lower = sb_cst32[:, C_LOWER:C_LOWER + S]
            sb_ones = sb_cst32[:, C_ONES:C_ONES + S]
            sb_bones = sb_cst32[:, C_BONES:C_BONES + BL]

            # ---- phase A: masks / positions / ranks (all 4 samples
            #      stacked on partitions: row 32*b + s) ------------------
            maskf = pp.tile([K, T], F32)        # [s-stacked, t] 0/1 mask
            counts = pp.tile([K, 1], F32)
            nc.vector.tensor_scalar(
                maskf[:], sb_idsbc[:], sb_steps[:], None, OP.is_equal, OP.add,
                accum_out=counts[:],
            )
            tm = pp.tile([K, T], F32)           # mask * (t - T)
            nc.vector.tensor_tensor(tm[:], maskf[:], sb_tmt[:], OP.mult)
            posm = pp.tile([K, 1], F32)         # pos - T (present) else 0
            nc.vector.tensor_reduce(posm[:], tm[:], mybir.AxisListType.X, OP.min)

            cnt1 = pp.tile([K, 1], F32)
            nc.vector.tensor_scalar(cnt1[:], counts[:], 1.0, None, OP.max)
            recip = pp.tile([K, 1], F32)        # 1 / max(counts, 1)
            nc.vector.reciprocal(recip[:], cnt1[:])

            # distinct sort keys: (pos-T)*33 + (s+1); order == stable
            # argsort of pos with id tiebreak (present strictly first)
            key = pp.tile([K, 1], F32)
            nc.vector.tensor_scalar(
                key[:], posm[:], 33.0, sb_steps[:], OP.mult, OP.add
            )
            key_sq = pp.tile([K, S], F32)
            nc.vector.tensor_scalar(key_sq[:], sb_ones[:], key[:], None, OP.mult)
            key_t = pp.tile([K, S], F32)        # row i holds key_l along l
            nc.vector.transpose(key_t[:], key_sq[:])
            cmp = pp.tile([K, S], F32)
            rank = pp.tile([K, 1], F32)
            nc.vector.tensor_scalar(
                cmp[:], key_t[:], key[:], None, OP.is_lt, OP.add,
                accum_out=rank[:],
            )
            rankp1 = pp.tile([K, 1], F32)
            nc.vector.tensor_scalar(rankp1[:], rank[:], 1.0, None, OP.add)
            t999 = pp.tile([K, 1], F32)         # 999 for absent steps
            nc.vector.tensor_scalar(
                t999[:], posm[:], 0.0, 999.0, OP.is_ge, OP.mult
            )
            rankp = pp.tile([K, 1], F32)        # rank, pushed out if absent
            nc.vector.tensor_tensor(rankp[:], rank[:], t999[:], OP.add)

            v_t = pp.tile([K, 8], F32)          # per-step stats columns
            nc.vector.tensor_scalar(v_t[:, 1:2], posm[:], 0.0, None, OP.is_lt)

            rankp_sq = pp.tile([K, S], F32)
            nc.vector.tensor_scalar(rankp_sq[:], sb_ones[:], rankp[:], None, OP.mult)
            rankp_t = pp.tile([K, S], F32)
            nc.vector.transpose(rankp_t[:], rankp_sq[:])
            rankp1_sq = pp.tile([K, S], F32)
            nc.vector.tensor_scalar(rankp1_sq[:], sb_ones[:], rankp1[:], None, OP.mult)
            rankp1_t = pp.tile([K, S], F32)
            nc.vector.transpose(rankp1_t[:], rankp1_sq[:])

            # A[i,j] = (rankp_j == rank_i + 1); succ_i = sum_j A[i,j]
            a_m = pp.tile([K, S], F32)
            nc.vector.tensor_scalar(
                a_m[:], rankp_t[:], rankp1[:], None, OP.is_equal, OP.add,
                accum_out=v_t[:, 0:1],
            )
            # A^T (lhsT for the H_next matmul; 0/1 so bf16 is exact)
            a_t = pp.tile([K, S], BF16)
            nc.vector.tensor_scalar(
                a_t[:], rankp1_t[:], rankp[:], None, OP.is_equal
            )
            # inv_i = sum_j A[i,j] * [i > j]
            a_inv = pp.tile([K, S], F32)
            nc.vector.scalar_tensor_tensor(
                a_inv[:], rankp_t[:], rankp1[:], sb_lower[:],
                op0=OP.is_equal, op1=OP.mult, accum_out=v_t[:, 2:3],
            )

            # ---- phase B: segment sums via TensorE --------------------
            h_all = pp.tile([K, D], BF16)
            hn = ps_misc.tile([K, D], F32)      # 2 PSUM banks
            diff = pp.tile([K, D], F32)
            sq = pp.tile([K, D], F32)
            e_raw = pp.tile([K, 1], F32)
            ps_of = {}

            # Per-sample tail, emitted one sample late (during sample b+1's
            # stream) so the ops land in each engine's in-order queue at a
            # point where their dependencies are already met — emitted
            # eagerly they head-of-line-block the abs stream and stall the
            # DMAs.  The H-scale half runs as early as possible because it
            # releases sample b's PSUM banks for sample b+2.
            def sample_scale(b):
                ps_all = ps_of[b]
                bs = slice(b * S, (b + 1) * S)
                for h in range(2):
                    nc.vector.tensor_scalar(
                        h_all[bs, h * 512:(h + 1) * 512],
                        ps_all[bs, h * 512:(h + 1) * 512],
                        recip[bs], None, OP.mult,
                    )

            def sample_tail(b):
                bs = slice(b * S, (b + 1) * S)
                for h in range(2):
                    nc.tensor.matmul(
                        hn[bs, h * 512:(h + 1) * 512],
                        lhsT=a_t[bs, :],
                        rhs=h_all[bs, h * 512:(h + 1) * 512],
                        start=True, stop=True,
                        tile_position=(b * S, b * S),
                    )
                nc.vector.tensor_tensor(
                    diff[bs, :], h_all[bs, :], hn[bs, :], OP.subtract
                )
                # relu(d)^2 == max(d,0)*d, with the free-dim sum fused in
                nc.vector.scalar_tensor_tensor(
                    sq[bs, :], diff[bs, :], 0.0, diff[bs, :],
                    op0=OP.max, op1=OP.mult, accum_out=e_raw[bs, :],
                )

            for b in range(BL):
                # all 16 mask chunks of the sample in one compare against
                # host-replicated ids (layout matches the x DMA below)
                mk_all = mkp.tile([K, NCHUNK * S], BF16)
                nc.vector.tensor_tensor(
                    mk_all[:],
                    sb_cst8[:, C8_IDSREP + b * NCHUNK * S:
                            C8_IDSREP + (b + 1) * NCHUNK * S],
                    sb_cst8[:, C8_IOTAT:C8_IOTAT + NCHUNK * S],
                    OP.is_equal,
                )
                # fresh PSUM banks per sample: sample b+1 accumulates while
                # sample b's H-scale still reads its own banks (no WAR)
                ps_all = ps_sums.tile([K, D], F32, tag="ps")
                ps_of[b] = ps_all
                for tq in range(NCHUNK // XT):
                    if tq == 1 and b > 0:
                        sample_scale(b - 1)
                    if tq == (NCHUNK // XT) // 2 and b > 0:
                        sample_tail(b - 1)
                    ti = b * (NCHUNK // XT) + tq
                    xt = xin.tile([K, XT, D], F32)
                    # All x DMAs go through the sync ring: the scalar ring's
                    # issue ops share the ACT sequencer with the abs
                    # ACTIVATEs, so a data-starved abs head-of-line-blocks
                    # later DMA issues and stalls the stream.  With 16 KiB
                    # of contiguous DRAM per partition (XT consecutive
                    # T-rows per partition; any (partition, sub) <-> t
                    # bijection works as long as the host ids layout
                    # matches), one ring's descriptor feed saturates all 16
                    # SDMA engines.
                    dma_eng = nc.sync
                    dma_eng.dma_start(
                        out=xt[:],
                        in_=x[b, tq * XT * K:(tq + 1) * XT * K, :].rearrange(
                            "(p s) d -> p s d", p=K
                        ),
                    )
                    # |x| rounded to bf16: the PE runs bf16 at 1 cycle/row
                    # vs fp32's 4; the 2^-9 relative rounding on |x| washes
                    # out to ~1e-4 in the final loss (mask stays exact 0/1).
                    # Alternate engines: ACT computes Abs->bf16 directly; DVE
                    # rounds to bf16 (RNE, so |bf16(x)| == bf16(|x|)) then
                    # clears the sign bit in place in the 16-bit 4x mode.
                    xa = xabs.tile([K, XT, D], BF16)
                    if ti % 2 == 0:
                        nc.scalar.activation(xa[:], xt[:], AF.Abs)
                    else:
                        nc.vector.tensor_copy(xa[:], xt[:])
                        nc.vector.tensor_scalar(
                            xa[:].bitcast(U16), xa[:].bitcast(U16),
                            0x7FFF, None, OP.bitwise_and,
                        )
                    for sub in range(XT):
                        c = tq * XT + sub
                        for h in range(2):
                            nc.tensor.matmul(
                                ps_all[b * S:(b + 1) * S, h * 512:(h + 1) * 512],
                                lhsT=mk_all[:, c * S:(c + 1) * S],
                                rhs=xa[:, sub, h * 512:(h + 1) * 512],
                                start=(c == 0), stop=(c == NCHUNK - 1),
                                tile_position=(0, b * S),
                            )
            sample_scale(BL - 1)
            sample_tail(BL - 1)

            # ---- phase C: combine per-step stats ----------------------
            e_col = pp.tile([K, 1], F32)
            nc.vector.tensor_scalar(e_col[:], e_raw[:], 1.0 / D, None, OP.mult)
            nc.vector.tensor_tensor(v_t[:, 3:4], e_col[:], v_t[:, 0:1], OP.mult)
            ae1 = pp.tile([K, 1], F32)          # relu(ALPHA - E)
            nc.vector.tensor_scalar(
                ae1[:], e_col[:], -1.0, ALPHA, OP.mult, OP.add
            )
            ae = pp.tile([K, 1], F32)
            nc.vector.tensor_scalar(ae[:], ae1[:], 0.0, None, OP.max)
            nc.vector.tensor_tensor(v_t[:, 4:5], ae[:], v_t[:, 2:3], OP.mult)

            # per-sample column sums: blockones[128,4].T @ V[128,5] -> [4,5]
            vp = ps_misc.tile([BL, 8], F32)
            nc.tensor.matmul(
                vp[:, 0:5], lhsT=sb_bones[:], rhs=v_t[:, 0:5],
                start=True, stop=True,
            )
            out_sb = pp.tile([BL, 5], F32)
            nc.vector.tensor_copy(out_sb[:], vp[:, 0:5])
            nc.sync.dma_start(out=out5[:], in_=out_sb[:])

    _split_sync_waits(nc)
    return nc


_PROGRAM: bass.Bass | None = None


def get_program() -> bass.Bass:
    global _PROGRAM
    if _PROGRAM is None:
        _PROGRAM = build_program()
    return _PROGRAM


def make_in_maps(inputs: np.ndarray, step_ids: np.ndarray) -> list[dict]:
    """Shard + pre-layout the (tiny) index tensors per core."""
    inputs = np.ascontiguousarray(np.asarray(inputs, dtype=np.float32))
    step_ids = np.asarray(step_ids)

    tmt16 = np.tile(
        (np.arange(T) - T).astype(np.int16)[None, :], (K, 1)
    )
    iota_t = np.tile(
        np.tile(np.arange(1, S + 1, dtype=np.int8), NCHUNK)[None, :], (K, 1)
    )
    cst32 = np.empty((K, CW32), dtype=np.float32)
    cst32[:, C_STEPS:C_STEPS + 1] = np.tile(
        np.arange(1, S + 1, dtype=np.float32), BL
    )[:, None]
    cst32[:, C_LOWER:C_LOWER + S] = np.tile(
        (np.arange(S)[:, None] > np.arange(S)[None, :]).astype(np.float32),
        (BL, 1),
    )
    cst32[:, C_ONES:C_ONES + S] = 1.0
    cst32[:, C_BONES:C_BONES + BL] = (
        (np.arange(K)[:, None] // S) == np.arange(BL)[None, :]
    ).astype(np.float32)

    in_maps = []
    for core in range(N_CORES):
        b0 = core * BL
        ids = step_ids[b0:b0 + BL].astype(np.int8)              # [4, 2048]
        # matmul chunk (b, tq, sub) contracts t = tq*XT*K + p*XT + sub on
        # partition p; idsrep repeats each id S times along the free dim so
        # one is_equal against iota_t yields all NCHUNK mask chunks
        idsrep = np.repeat(
            ids.reshape(BL, NCHUNK // XT, K, XT).transpose(2, 0, 1, 3)
            .reshape(K, BL, NCHUNK),
            S, axis=2,
        ).reshape(K, BL * NCHUNK * S)
        cst8 = np.empty((K, CW8), dtype=np.int8)
        cst8[:, C8_IDSBC:C8_IDSBC + T] = np.repeat(ids, S, axis=0)
        cst8[:, C8_IDSREP:C8_IDSREP + BL * NCHUNK * S] = idsrep
        cst8[:, C8_IOTAT:C8_IOTAT + NCHUNK * S] = iota_t
        in_maps.append({
            "x": inputs[b0:b0 + BL],
            "cst8": cst8,
            "tmt16": tmt16,
            "cst32": cst32,
        })
    return in_maps


def finish_host(out5_per_core: list[np.ndarray], binary_labels: np.ndarray):
    """Combine per-sample (npairs, n, ninv, S1, S2) with labels."""
    v = np.concatenate([np.asarray(o, np.float64) for o in out5_per_core], axis=0)
    npairs, n, ninv, s1, s2 = v[:, 0], v[:, 1], v[:, 2], v[:, 3], v[:, 4]
    labels = np.asarray(binary_labels)
    loss_pos = s1 / np.maximum(npairs, 1.0)
    loss_neg = s2 / np.maximum(ninv, 1.0)
    pos_count = (labels == 1) & (n >= 2)
    neg_count = (labels == 0) & (ninv > 0)
    total = (loss_pos * pos_count).sum() + (loss_neg * neg_count).sum()
    num = pos_count.sum() + neg_count.sum()
    return np.float32(total / (num + 1e-9))


def kernel(inputs, step_ids, binary_labels, _trace=False):
    nc = get_program()
    in_maps = make_in_maps(inputs, step_ids)
    res = run_bass_kernel_spmd(
        nc, in_maps, core_ids=list(range(N_CORES)), trace=_trace
    )
    out = finish_host([r["out5"] for r in res.results], binary_labels)
    if _trace:
        return out, res
    return out


# revision 53
# speedup vs baseline: 1.2537x; 1.1331x over previous
"""Trainium2 Bass kernel for nn_MaxMarginLoss (segment_reduce).

Data-parallel over the batch: 32 samples -> 8 NeuronCores x 4 samples.

Per core, for each sample b:
  - segment sums over T=2048 timesteps into S=32 step buckets are computed
    on TensorE as mask[128t,32s].T @ |x|[128t,1024d], accumulated over 16
    K-chunks into PSUM (this is the memory-bound part: 32 MiB of `inputs`
    per core, streamed as 2 MiB contiguous DMAs).
  - the appearance-order logic avoids any sort: first-appearance positions
    come from a masked min-reduce; each step's rank is the count of
    strictly-smaller packed keys (pos*33 + id); the ordered-adjacency
    matrix A[i,j] = (rank_j == rank_i + 1 and j present) turns "gather by
    argsort and diff neighbours" into a tiny 32x32 matmul H_next = A @ H.
  - pair energies E_i = mean_d relu(H_i - H_next_i)^2 via Relu + Square
    with fused free-dim accumulation.
Each core returns [4,5] per-sample sums (npairs, n, ninv, sum E*valid,
sum relu(1-E)*inv); the host applies the binary labels and the final
scalar division (a few hundred flops).
"""

import numpy as np

import concourse.bass as bass
from concourse import mybir
from concourse.bass_utils import run_bass_kernel_spmd
from concourse.tile import TileContext
from concourse.vector_clock import ScopedClock

F32 = mybir.dt.float32
BF16 = mybir.dt.bfloat16
U32 = mybir.dt.uint32
U16 = mybir.dt.uint16
I8 = mybir.dt.int8
I16 = mybir.dt.int16
OP = mybir.AluOpType
AF = mybir.ActivationFunctionType

B, T, D = 32, 2048, 1024
S = 32          # step ids 1..32; id 0 is padding
ALPHA = 1.0
N_CORES = 8
BL = B // N_CORES           # samples per core
K = 128                     # matmul contraction tile (partitions)
NCHUNK = T // K             # 16 K-chunks per sample
XT = 2                      # K-chunks per x DMA ([128, XT, 1024] = 1 MiB)

# The public neuronxcc walrus (setupSyncWait in CoreV2/V3GenImpl) only
# supports a small number of embedded semaphore waits per instruction,
# while Tile's scheduler attaches one wait per required logical proc.
# After scheduling, hoist overflow waits onto same-engine no-ops placed
# immediately before the owning instruction: engine program order makes
# that semantically identical.
_MAX_WAITS_DEFAULT = 1
_MAX_WAITS_BY_OPCODE = {}


class _LeanTailTileContext(TileContext):
    """Tile's default kernel tail is drain -> barrier -> sem-clear ->
    barrier.  After the first all-engine barrier no engine can still be
    waiting on a kernel semaphore, so the clears need no cross-engine
    ordering and the second (~3-4 us) barrier can be dropped; each
    engine's stream still ends after its own clears, so re-execution
    sees zeroed semaphores."""

    def _drain_and_barrier(self, tick_clock, wait_clock):
        drain_inst = self.nc.sync.drain()
        wait_clock.add_sem_waits(
            drain_inst.ins, ScopedClock({None: tick_clock.global_clock})
        )
        self.nc.all_engine_barrier()
        assert self.sems is not None
        popped = self.nc._tile_sem_poison_stack.pop()
        assert popped is self._sem_poison
        self.nc.clear_and_free_semaphores(list(self.sems.allocated().values()))


def _split_sync_waits(nc: bass.Bass):
    for f in nc.m.functions:
        for bb in f.blocks:
            insts = list(bb.instructions)
            need = []  # (ins, overflow_waits)
            for ins in insts:
                si = getattr(ins, "sync_info", None)
                if si is None or not si.on_wait:
                    continue
                cap = _MAX_WAITS_BY_OPCODE.get(ins.opcode, _MAX_WAITS_DEFAULT)
                waits = list(si.on_wait)
                if len(waits) <= cap:
                    continue
                ins.sync_info = mybir.SyncInfo(
                    on_wait=waits[:cap], on_update=list(si.on_update)
                )
                need.append((ins, waits[cap:], cap))
            if not need:
                continue
            nop_for: dict[str, list] = {}
            for ins, overflow, cap in need:
                eng = nc.engines[ins.engine]
                nops = []
                for i in range(0, len(overflow), cap):
                    nop = eng.nop(hint="waitsplit", nofuse=True)
                    nop.ins.sync_info = mybir.SyncInfo(
                        on_wait=overflow[i:i + cap], on_update=[]
                    )
                    nops.append(nop.ins)
                nop_for[ins.name] = nops
            created = {n.name for nops in nop_for.values() for n in nops}
            # nop() appended the new instructions to the current bb; pull
            # them out of every block and splice before their owners.
            for bb2 in f.blocks:
                cur = [i for i in bb2.instructions if i.name not in created]
                out = []
                for ins in cur:
                    out.extend(nop_for.get(ins.name, ()))
                    out.append(ins)
                bb2.instructions = out


# column offsets inside the packed int8 index buffer "cst8"
C8_IDSBC = 0                  # [K, T]   step ids, row 32b+s = ids of sample b
C8_IDSREP = C8_IDSBC + T      # [K, BL*NCHUNK*S] mask-layout ids, 32x repeated
C8_IOTAT = C8_IDSREP + BL * NCHUNK * S   # [K, NCHUNK*S] tiled 1..32
CW8 = C8_IOTAT + NCHUNK * S
# column offsets inside the packed float32 constant buffer "cst32"
C_STEPS = 0                   # [K, 1]
C_LOWER = C_STEPS + 1         # [K, S] block [i > j]
C_ONES = C_LOWER + S          # [K, S] ones
C_BONES = C_ONES + S          # [K, BL] block-diagonal ones
CW32 = C_BONES + BL


def build_program() -> bass.Bass:
    nc = bass.Bass()

    x = nc.declare_dram_parameter("x", [BL, T, D], F32, isOutput=False)
    cst8 = nc.declare_dram_parameter("cst8", [K, CW8], I8, isOutput=False)
    tmt16 = nc.declare_dram_parameter("tmt16", [K, T], I16, isOutput=False)
    cst32 = nc.declare_dram_parameter("cst32", [K, CW32], F32, isOutput=False)
    out5 = nc.declare_dram_parameter("out5", [BL, 5], F32, isOutput=True)

    with _LeanTailTileContext(nc) as tc:
        with (
            tc.tile_pool(name="const", bufs=1) as cpool,
            tc.tile_pool(name="persist", bufs=1) as pp,
            tc.tile_pool(name="xin", bufs=12) as xin,
            tc.tile_pool(name="xabs", bufs=12) as xabs,
            tc.tile_pool(name="mk", bufs=2) as mkp,
            tc.tile_pool(name="ps_sums", bufs=2, space="PSUM") as ps_sums,
            tc.tile_pool(name="ps_misc", bufs=1, space="PSUM") as ps_misc,
        ):
            # ---- constants / index data, narrow dtypes, DMA'd via the
            #      (otherwise idle) SWDGE queue so the HWDGE rings start
            #      on x immediately --------------------------------------
            sb_cst8 = cpool.tile([K, CW8], I8)
            nc.gpsimd.dma_start(out=sb_cst8[:], in_=cst8[:])
            sb_tmt = cpool.tile([K, T], I16)
            nc.gpsimd.dma_start(out=sb_tmt[:], in_=tmt16[:])
            sb_cst32 = cpool.tile([K, CW32], F32)
            nc.gpsimd.dma_start(out=sb_cst32[:], in_=cst32[:])
            sb_idsbc = sb_cst8[:, C8_IDSBC:C8_IDSBC + T]
            sb_steps = sb_cst32[:, C_STEPS:C_STEPS + 1]
            sb_lower = sb_cst32[:, C_LOWER:C_LOWER + S]
            sb_ones = sb_cst32[:, C_ONES:C_ONES + S]
            sb_bones = sb_cst32[:, C_BONES:C_BONES + BL]

            # ---- phase A: masks / positions / ranks (all 4 samples
            #      stacked on partitions: row 32*b + s) ------------------
            maskf = pp.tile([K, T], F32)        # [s-stacked, t] 0/1 mask
            counts = pp.tile([K, 1], F32)
            nc.vector.tensor_scalar(
                maskf[:], sb_idsbc[:], sb_steps[:], None, OP.is_equal, OP.add,
                accum_out=counts[:],
            )
            tm = pp.tile([K, T], F32)           # mask * (t - T)
            nc.vector.tensor_tensor(tm[:], maskf[:], sb_tmt[:], OP.mult)
            posm = pp.tile([K, 1], F32)         # pos - T (present) else 0
            nc.vector.tensor_reduce(posm[:], tm[:], mybir.AxisListType.X, OP.min)

            cnt1 = pp.tile([K, 1], F32)
            nc.vector.tensor_scalar(cnt1[:], counts[:], 1.0, None, OP.max)
            recip = pp.tile([K, 1], F32)        # 1 / max(counts, 1)
            nc.vector.reciprocal(recip[:], cnt1[:])

            # distinct sort keys: (pos-T)*33 + (s+1); order == stable
            # argsort of pos with id tiebreak (present strictly first)
            key = pp.tile([K, 1], F32)
            nc.vector.tensor_scalar(
                key[:], posm[:], 33.0, sb_steps[:], OP.mult, OP.add
            )
            key_sq = pp.tile([K, S], F32)
            nc.vector.tensor_scalar(key_sq[:], sb_ones[:], key[:], None, OP.mult)
            key_t = pp.tile([K, S], F32)        # row i holds key_l along l
            nc.vector.transpose(key_t[:], key_sq[:])
            cmp = pp.tile([K, S], F32)
            rank = pp.tile([K, 1], F32)
            nc.vector.tensor_scalar(
                cmp[:], key_t[:], key[:], None, OP.is_lt, OP.add,
                accum_out=rank[:],
            )
            rankp1 = pp.tile([K, 1], F32)
            nc.vector.tensor_scalar(rankp1[:], rank[:], 1.0, None, OP.add)
            t999 = pp.tile([K, 1], F32)         # 999 for absent steps
            nc.vector.tensor_scalar(
                t999[:], posm[:], 0.0, 999.0, OP.is_ge, OP.mult
            )
            rankp = pp.tile([K, 1], F32)        # rank, pushed out if absent
            nc.vector.tensor_tensor(rankp[:], rank[:], t999[:], OP.add)

            v_t = pp.tile([K, 8], F32)          # per-step stats columns
            nc.vector.tensor_scalar(v_t[:, 1:2], posm[:], 0.0, None, OP.is_lt)

            rankp_sq = pp.tile([K, S], F32)
            nc.vector.tensor_scalar(rankp_sq[:], sb_ones[:], rankp[:], None, OP.mult)
            rankp_t = pp.tile([K, S], F32)
            nc.vector.transpose(rankp_t[:], rankp_sq[:])
            rankp1_sq = pp.tile([K, S], F32)
            nc.vector.tensor_scalar(rankp1_sq[:], sb_ones[:], rankp1[:], None, OP.mult)
            rankp1_t = pp.tile([K, S], F32)
            nc.vector.transpose(rankp1_t[:], rankp1_sq[:])

            # A[i,j] = (rankp_j == rank_i + 1); succ_i = sum_j A[i,j]
            a_m = pp.tile([K, S], F32)
            nc.vector.tensor_scalar(
                a_m[:], rankp_t[:], rankp1[:], None, OP.is_equal, OP.add,
                accum_out=v_t[:, 0:1],
            )
            # A^T (lhsT for the H_next matmul; 0/1 so bf16 is exact)
            a_t = pp.tile([K, S], BF16)
            nc.vector.tensor_scalar(
                a_t[:], rankp1_t[:], rankp[:], None, OP.is_equal
            )
            # inv_i = sum_j A[i,j] * [i > j]
            a_inv = pp.tile([K, S], F32)
            nc.vector.scalar_tensor_tensor(
                a_inv[:], rankp_t[:], rankp1[:], sb_lower[:],
                op0=OP.is_equal, op1=OP.mult, accum_out=v_t[:, 2:3],
            )

            # ---- phase B: segment sums via TensorE --------------------
            h_all = pp.tile([K, D], BF16)
            hn = ps_misc.tile([K, D], F32)      # 2 PSUM banks
            diff = pp.tile([K, D], F32)
            sq = pp.tile([K, D], F32)
            e_raw = pp.tile([K, 1], F32)
            ps_of = {}

            # Per-sample tail, emitted one sample late (during sample b+1's
            # stream) so the ops land in each engine's in-order queue at a
            # point where their dependencies are already met — emitted
            # eagerly they head-of-line-block the abs stream and stall the
            # DMAs.  The H-scale half runs as early as possible because it
            # releases sample b's PSUM banks for sample b+2.
            def sample_scale(b):
                ps_all = ps_of[b]
                bs = slice(b * S, (b + 1) * S)
                for h in range(2):
                    nc.vector.tensor_scalar(
                        h_all[bs, h * 512:(h + 1) * 512],
                        ps_all[bs, h * 512:(h + 1) * 512],
                        recip[bs], None, OP.mult,
                    )

            def sample_tail(b):
                bs = slice(b * S, (b + 1) * S)
                for h in range(2):
                    nc.tensor.matmul(
                        hn[bs, h * 512:(h + 1) * 512],
                        lhsT=a_t[bs, :],
                        rhs=h_all[bs, h * 512:(h + 1) * 512],
                        start=True, stop=True,
                        tile_position=(b * S, b * S),
                    )
                nc.vector.tensor_tensor(
                    diff[bs, :], h_all[bs, :], hn[bs, :], OP.subtract
                )
                # relu(d)^2 == max(d,0)*d, with the free-dim sum fused in
                nc.vector.scalar_tensor_tensor(
                    sq[bs, :], diff[bs, :], 0.0, diff[bs, :],
                    op0=OP.max, op1=OP.mult, accum_out=e_raw[bs, :],
                )

            for b in range(BL):
                # all 16 mask chunks of the sample in one compare against
                # host-replicated ids (layout matches the x DMA below)
                mk_all = mkp.tile([K, NCHUNK * S], BF16)
                nc.vector.tensor_tensor(
                    mk_all[:],
                    sb_cst8[:, C8_IDSREP + b * NCHUNK * S:
                            C8_IDSREP + (b + 1) * NCHUNK * S],
                    sb_cst8[:, C8_IOTAT:C8_IOTAT + NCHUNK * S],
                    OP.is_equal,
                )
                # fresh PSUM banks per sample: sample b+1 accumulates while
                # sample b's H-scale still reads its own banks (no WAR)
                ps_all = ps_sums.tile([K, D], F32, tag="ps")
                ps_of[b] = ps_all
                for tq in range(NCHUNK // XT):
                    if tq == 1 and b > 0:
                        sample_scale(b - 1)
                    if tq == (NCHUNK // XT) // 2 and b > 0:
                        sample_tail(b - 1)
                    ti = b * (NCHUNK // XT) + tq
                    xt = xin.tile([K, XT, D], F32)
                    # All x DMAs go through the sync ring: the scalar ring's
                    # issue ops share the ACT sequencer with the abs
                    # ACTIVATEs, so a data-starved abs head-of-line-blocks
                    # later DMA issues and stalls the stream.  With 16 KiB
                    # of contiguous DRAM per partition (XT consecutive
                    # T-rows per partition; any (partition, sub) <-> t
                    # bijection works as long as the host ids layout
                    # matches), one ring's descriptor feed saturates all 16
                    # SDMA engines.
                    dma_eng = nc.sync
                    dma_eng.dma_start(
                        out=xt[:],
                        in_=x[b, tq * XT * K:(tq + 1) * XT * K, :].rearrange(
                            "(p s) d -> p s d", p=K
                        ),
                    )
                    # |x| rounded to bf16: the PE runs bf16 at 1 cycle/row
                    # vs fp32's 4; the 2^-9 relative rounding on |x| washes
                    # out to ~1e-4 in the final loss (mask stays exact 0/1).
                    # Alternate engines: ACT computes Abs->bf16 directly; DVE
                    # rounds to bf16 (RNE, so |bf16(x)| == bf16(|x|)) then
                    # clears the sign bit in place in the 16-bit 4x mode.
                    xa = xabs.tile([K, XT, D], BF16)
                    if ti % 2 == 0:
                        nc.scalar.activation(xa[:], xt[:], AF.Abs)
                    else:
                        nc.vector.tensor_copy(xa[:], xt[:])
                        nc.vector.tensor_scalar(
                            xa[:].bitcast(U16), xa[:].bitcast(U16),
                            0x7FFF, None, OP.bitwise_and,
                        )
                    for sub in range(XT):
                        c = tq * XT + sub
                        for h in range(2):
                            nc.tensor.matmul(
                                ps_all[b * S:(b + 1) * S, h * 512:(h + 1) * 512],
                                lhsT=mk_all[:, c * S:(c + 1) * S],
                                rhs=xa[:, sub, h * 512:(h + 1) * 512],
                                start=(c == 0), stop=(c == NCHUNK - 1),
                                tile_position=(0, b * S),
                            )
            sample_scale(BL - 1)
            sample_tail(BL - 1)

            # ---- phase C: combine per-step stats ----------------------
            e_col = pp.tile([K, 1], F32)
            nc.vector.tensor_scalar(e_col[:], e_raw[:], 1.0 / D, None, OP.mult)
            nc.vector.tensor_tensor(v_t[:, 3:4], e_col[:], v_t[:, 0:1], OP.mult)
            ae1 = pp.tile([K, 1], F32)          # relu(ALPHA - E)
            nc.vector.tensor_scalar(
                ae1[:], e_col[:], -1.0, ALPHA, OP.mult, OP.add
            )
            ae = pp.tile([K, 1], F32)
            nc.vector.tensor_scalar(ae[:], ae1[:], 0.0, None, OP.max)
            nc.vector.tensor_tensor(v_t[:, 4:5], ae[:], v_t[:, 2:3], OP.mult)

            # per-sample column sums: blockones[128,4].T @ V[128,5] -> [4,5]
            vp = ps_misc.tile([BL, 8], F32)
            nc.tensor.matmul(
                vp[:, 0:5], lhsT=sb_bones[:], rhs=v_t[:, 0:5],
                start=True, stop=True,
            )
            out_sb = pp.tile([BL, 5], F32)
            nc.vector.tensor_copy(out_sb[:], vp[:, 0:5])
            nc.sync.dma_start(out=out5[:], in_=out_sb[:])

    _split_sync_waits(nc)
    return nc


_PROGRAM: bass.Bass | None = None


def get_program() -> bass.Bass:
    global _PROGRAM
    if _PROGRAM is None:
        _PROGRAM = build_program()
    return _PROGRAM


def make_in_maps(inputs: np.ndarray, step_ids: np.ndarray) -> list[dict]:
    """Shard + pre-layout the (tiny) index tensors per core."""
    inputs = np.ascontiguousarray(np.asarray(inputs, dtype=np.float32))
    step_ids = np.asarray(step_ids)

    tmt16 = np.tile(
        (np.arange(T) - T).astype(np.int16)[None, :], (K, 1)
    )
    iota_t = np.tile(
        np.tile(np.arange(1, S + 1, dtype=np.int8), NCHUNK)[None, :], (K, 1)
    )
    cst32 = np.empty((K, CW32), dtype=np.float32)
    cst32[:, C_STEPS:C_STEPS + 1] = np.tile(
        np.arange(1, S + 1, dtype=np.float32), BL
    )[:, None]
    cst32[:, C_LOWER:C_LOWER + S] = np.tile(
        (np.arange(S)[:, None] > np.arange(S)[None, :]).astype(np.float32),
        (BL, 1),
    )
    cst32[:, C_ONES:C_ONES + S] = 1.0
    cst32[:, C_BONES:C_BONES + BL] = (
        (np.arange(K)[:, None] // S) == np.arange(BL)[None, :]
    ).astype(np.float32)

    in_maps = []
    for core in range(N_CORES):
        b0 = core * BL
        ids = step_ids[b0:b0 + BL].astype(np.int8)              # [4, 2048]
        # matmul chunk (b, tq, sub) contracts t = tq*XT*K + p*XT + sub on
        # partition p; idsrep repeats each id S times along the free dim so
        # one is_equal against iota_t yields all NCHUNK mask chunks
        idsrep = np.repeat(
            ids.reshape(BL, NCHUNK // XT, K, XT).transpose(2, 0, 1, 3)
            .reshape(K, BL, NCHUNK),
            S, axis=2,
        ).reshape(K, BL * NCHUNK * S)
        cst8 = np.empty((K, CW8), dtype=np.int8)
        cst8[:, C8_IDSBC:C8_IDSBC + T] = np.repeat(ids, S, axis=0)
        cst8[:, C8_IDSREP:C8_IDSREP + BL * NCHUNK * S] = idsrep
        cst8[:, C8_IOTAT:C8_IOTAT + NCHUNK * S] = iota_t
        in_maps.append({
            "x": inputs[b0:b0 + BL],
            "cst8": cst8,
            "tmt16": tmt16,
            "cst32": cst32,
        })
    return in_maps


def finish_host(out5_per_core: list[np.ndarray], binary_labels: np.ndarray):
    """Combine per-sample (npairs, n, ninv, S1, S2) with labels."""
    v = np.concatenate([np.asarray(o, np.float64) for o in out5_per_core], axis=0)
    npairs, n, ninv, s1, s2 = v[:, 0], v[:, 1], v[:, 2], v[:, 3], v[:, 4]
    labels = np.asarray(binary_labels)
    loss_pos = s1 / np.maximum(npairs, 1.0)
    loss_neg = s2 / np.maximum(ninv, 1.0)
    pos_count = (labels == 1) & (n >= 2)
    neg_count = (labels == 0) & (ninv > 0)
    total = (loss_pos * pos_count).sum() + (loss_neg * neg_count).sum()
    num = pos_count.sum() + neg_count.sum()
    return np.float32(total / (num + 1e-9))


def kernel(inputs, step_ids, binary_labels, _trace=False):
    nc = get_program()
    in_maps = make_in_maps(inputs, step_ids)
    res = run_bass_kernel_spmd(
        nc, in_maps, core_ids=list(range(N_CORES)), trace=_trace
    )
    out = finish_host([r["out5"] for r in res.results], binary_labels)
    if _trace:
        return out, res
    return out
